# revision 4
# baseline (speedup 1.0000x reference)
"""Trainium2 Bass kernel for BertAlibiUnpadSelfAttention.

Problem shapes (hardcoded): B=2, S=2048, H=12, D=64, DIM=768.
Reference computation:
    qkv = hidden @ Wqkv_w.T + Wqkv_b            # (4096, 2304)
    pad via indices (a permutation -> pure row shuffle)
    q,k,v = split/reshape -> (b, h, s, d)
    scores = q @ k.T / sqrt(64) + bias          # bias dense (2,12,2048,2048)
    attn = softmax(scores) @ v -> (4096, 768), unpad via indices

Sharding: 24 (batch, head) pairs -> 3 per core across 8 cores. Each core
computes its own slice of the QKV projection (disjoint columns/rows -> no
redundant FLOPs) and full attention for its 3 heads.

Device kernel layout choices (v3 - engine-balanced softmax + LAG pipeline):
  - qT/kT computed in [d, s] layout directly (lhsT = W slices, rhs = hidden^T),
    which is exactly the layout the scores matmul wants.  The Q side is
    pre-scaled by A/sqrt(D) with A = 2^10/ln2, so PSUM scores are s*A.
  - scores are computed TRANSPOSED: scoresT[sk, sq] tiles, so the softmax
    reduction (over sk) can be done by the PV matmul itself: V gets an
    appended ones-column, so PV produces [attnT ; sums] in one accumulation.
  - exp() is SPLIT across two engines by query column to balance load:
      cols 0:AW of each 1024-unit (path A): ScalarE ACTIVATE Exp
        (scale=1/A) -> fp16, then VectorE multiply by exp(bias) fp16 (2x).
      cols AW:1024 (path B): single VectorE tensor_tensor: int16(round(
        s*A + b*A + 15360 - C)) whose BITS are the fp16 Schraudolph
        approximation of exp(s+b) (+-4% sawtooth, zero mean log error with
        C=59.65; averages to ~0.1% after the PV reduction).
    The split is a pure query-column split, so each softmax row (fixed
    query, all keys) is handled by exactly one path and systematic factors
    cancel in the normalization.
  - PV matmuls are issued LAG units behind QK/evac (software pipelining),
    so the in-order PE queue never stalls waiting for the current unit's
    evacuation -> PE stays dense -> HAM clock gate stays at 8/8.
  - Final normalize (divide by sums) + transpose back to [s, d] + V-bias add
    happen on the host (tiny: 3x65x2048 per core).
"""

import math
import numpy as np

B, S, H, D = 2, 2048, 12, 64
DIM = H * D            # 768
TOTAL = B * S          # 4096
HPC = 3                # heads per core
N_CORES = 8
KT = DIM // 128        # 6 k-tiles of 128
SQC = S // 512         # 4 free-dim chunks of 512
SKT = S // 128         # 16 sk tiles of 128
VST = HPC * 65         # vp cols per st block: [h0 64 + one | h1 ... | h2 ...]

A_EXP = 1024.0 / math.log(2.0)   # fp16 Schraudolph scale, 1477.32
C_OPT = 59.65                    # zero-mean-log correction
B_OFF = 15360.0 - C_OPT          # 15*1024 - C
AW = 704                         # path-A width per 1024-col unit
LAG = 2                          # PV issue lag (units) behind QK/evac
PRE = 3                          # bias DMA prefetch depth (st tiles)

_CACHE = {}


def _build_nc():
    """Build + compile the per-core Bass module (fp16 operands, fp32 PSUM)."""
    from concourse import bacc, mybir, tile

    f32 = mybir.dt.float32
    f16 = mybir.dt.float16
    i16 = mybir.dt.int16

    nc = bacc.Bacc("TRN2", target_bir_lowering=False, debug=False)

    hT = nc.dram_tensor("hT", (DIM, S), f16, kind="ExternalInput")
    wq = nc.dram_tensor("wq", (DIM, HPC * D), f16, kind="ExternalInput")
    wk = nc.dram_tensor("wk", (DIM, HPC * D), f16, kind="ExternalInput")
    wv = nc.dram_tensor("wv", (DIM, HPC * D), f16, kind="ExternalInput")
    bq = nc.dram_tensor("bq", (HPC * D, 1), f32, kind="ExternalInput")
    bk = nc.dram_tensor("bk", (HPC * D, 1), f32, kind="ExternalInput")
    # per head [sk, sq]; in each 1024-col half: cols 0:AW fp16 exp(bias)
    # bits, cols AW:1024 int16 round(bias*A + B_OFF)
    bias_t = nc.dram_tensor("bias_t", (HPC, S, S), i16, kind="ExternalInput")
    out = nc.dram_tensor("out", (HPC, D + 1, S), f32, kind="ExternalOutput")

    EXP = mybir.ActivationFunctionType.Exp
    IDENT = mybir.ActivationFunctionType.Identity
    ADD = mybir.AluOpType.add

    with tile.TileContext(nc) as tc:
        with (
            tc.tile_pool(name="const", bufs=1) as constp,
            tc.tile_pool(name="bias", bufs=2 * PRE + 2) as biasp,
            tc.tile_pool(name="pt", bufs=LAG + 4) as ptp,
            tc.tile_pool(name="ot", bufs=3) as otp,
        ):
            # ---- load persistent inputs ----
            ht = [constp.tile([128, S], f16, tag=f"ht{i}", name=f"ht{i}") for i in range(KT)]
            for i in range(KT):
                nc.sync.dma_start(ht[i][:], hT[i * 128:(i + 1) * 128, :])

            wq_sb = [constp.tile([128, HPC * D], f16, tag=f"wq{i}", name=f"wq{i}") for i in range(KT)]
            wk_sb = [constp.tile([128, HPC * D], f16, tag=f"wk{i}", name=f"wk{i}") for i in range(KT)]
            wv_sb = [constp.tile([128, HPC * D], f16, tag=f"wv{i}", name=f"wv{i}") for i in range(KT)]
            bq_sb = constp.tile([128, 1], f32, tag="bq0")
            bq_sb2 = constp.tile([64, 1], f32, tag="bq1")
            bk_sb = constp.tile([128, 1], f32, tag="bk0")
            bk_sb2 = constp.tile([64, 1], f32, tag="bk1")
            nc.scalar.dma_start(bq_sb[:], bq[0:128, :])
            nc.scalar.dma_start(bq_sb2[:], bq[128:192, :])
            nc.scalar.dma_start(bk_sb[:], bk[0:128, :])
            nc.scalar.dma_start(bk_sb2[:], bk[128:192, :])
            for i in range(KT):
                nc.scalar.dma_start(wq_sb[i][:], wq[i * 128:(i + 1) * 128, :])
                nc.scalar.dma_start(wk_sb[i][:], wk[i * 128:(i + 1) * 128, :])
                nc.scalar.dma_start(wv_sb[i][:], wv[i * 128:(i + 1) * 128, :])
            # Q/K in [d, s] layout: heads 0,1 in tile0 (partitions 0-63 /
            # 64-127), head 2 in tile1 (partitions 0-63).  Same base
            # partition for q_j and k_j so the scores matmul operands align.
            q0 = constp.tile([128, S], f16, tag="q0")
            q1 = constp.tile([64, S], f16, tag="q1")
            k0 = constp.tile([128, S], f16, tag="k0")
            k1 = constp.tile([64, S], f16, tag="k1")
            # V' blocks per st: [h0 d0..63, one, h1 d0..63, one, h2 ...];
            # the ones come from the memset and give the softmax row-sums.
            vp = constp.tile([128, SKT * VST], f16, tag="vp")
            nc.vector.memset(vp[:], 1.0)

            # ---- phase 1a: qT / kT projection (+ bias via ScalarE) ----
            with tc.tile_pool(name="psA", bufs=2, space="PSUM") as psA:
                for (dst, wsb, bsb, col0, m) in (
                    (q0, wq_sb, bq_sb, 0, 128),
                    (q1, wq_sb, bq_sb2, 128, 64),
                    (k0, wk_sb, bk_sb, 0, 128),
                    (k1, wk_sb, bk_sb2, 128, 64),
                ):
                    for c in range(SQC):
                        ps = psA.tile([m, 512], f32, tag=f"psA{m}", name=f"psA{m}")
                        for i in range(KT):
                            nc.tensor.matmul(
                                ps[:],
                                wsb[i][:, col0:col0 + m],
                                ht[i][:, c * 512:(c + 1) * 512],
                                start=(i == 0), stop=(i == KT - 1),
                            )
                        nc.scalar.activation(
                            dst[:, c * 512:(c + 1) * 512], ps[:], IDENT,
                            bias=bsb[:])

                # ---- phase 1b: V in natural [s, d] layout ----
                for st in range(SKT):
                    psv = psA.tile([128, HPC * D], f32, tag="psV", name="psV")
                    for i in range(KT):
                        nc.tensor.matmul(
                            psv[:],
                            ht[i][:, st * 128:(st + 1) * 128],
                            wv_sb[i][:],
                            start=(i == 0), stop=(i == KT - 1),
                        )
                    for j in range(HPC):
                        nc.vector.tensor_copy(
                            vp[:, st * VST + j * 65: st * VST + j * 65 + D],
                            psv[:, j * D:(j + 1) * D])

            # ---- phase 2: attention, software-pipelined over 96 units ----
            qk_slices = (  # (q_ap, k_ap) per head, matching base partitions
                (q0[0:64, :], k0[0:64, :]),
                (q0[64:128, :], k0[64:128, :]),
                (q1[:, :], k1[:, :]),
            )
            NU = HPC * SKT * 2   # 96 units of [128 sk, 1024 sq]
            bt_tiles = {}        # flat st index (j*SKT+st) -> bias tile
            pt_tiles = {}        # unit index -> pt tile
            po_tiles = {}        # head -> [po chunks]

            def issue_bias_dma(si):
                if si >= HPC * SKT:
                    return
                j, st = divmod(si, SKT)
                bt = biasp.tile([128, S], i16, tag="bt", name=f"bt{si}")
                nc.sync.dma_start(bt[:], bias_t[j, st * 128:(st + 1) * 128, :])
                bt_tiles[si] = bt

            with (
                tc.tile_pool(name="ps", bufs=2, space="PSUM") as psp,
                tc.tile_pool(name="po", bufs=4, space="PSUM") as pop,
            ):
                for si in range(PRE):
                    issue_bias_dma(si)

                for u in range(NU + LAG):
                    if u < NU:
                        j, r = divmod(u, 2 * SKT)
                        st, half = divmod(r, 2)
                        si = j * SKT + st
                        if half == 0:
                            issue_bias_dma(si + PRE)
                        qap, kap = qk_slices[j]
                        bt = bt_tiles[si]
                        ps = psp.tile([128, 1024], f32, tag="ps", name=f"ps{u}")
                        pt = ptp.tile([128, 1024], f16, tag="pt", name=f"pt{u}")
                        pt_tiles[u] = pt
                        for cc in range(2):
                            c = half * 2 + cc
                            nc.tensor.matmul(
                                ps[:, cc * 512:(cc + 1) * 512],
                                kap[:, st * 128:(st + 1) * 128],
                                qap[:, c * 512:(c + 1) * 512],
                                start=True, stop=True,
                            )
                        hb = half * 1024
                        # path A: exact exp on ScalarE, * exp(bias) on DVE
                        nc.scalar.activation(
                            pt[:, 0:AW], ps[:, 0:AW], EXP, scale=1.0 / A_EXP)
                        nc.vector.tensor_mul(
                            pt[:, 0:AW], pt[:, 0:AW],
                            bt[:, hb:hb + AW].bitcast(f16))
                        # path B: Schraudolph exp via int16 convert
                        nc.vector.tensor_tensor(
                            pt[:, AW:1024].bitcast(i16), ps[:, AW:1024],
                            bt[:, hb + AW:hb + 1024], ADD)
                    up = u - LAG
                    if up >= 0:
                        jp, rp = divmod(up, 2 * SKT)
                        stp, halfp = divmod(rp, 2)
                        if rp == 0:
                            po_tiles[jp] = [
                                pop.tile([D + 1, 512], f32, tag="po",
                                         name=f"po{jp}_{_c}")
                                for _c in range(SQC)]
                        po = po_tiles[jp]
                        ptp_ = pt_tiles.pop(up)
                        for cc in range(2):
                            c = halfp * 2 + cc
                            nc.tensor.matmul(
                                po[c][:],
                                vp[:, stp * VST + jp * 65: stp * VST + jp * 65 + D + 1],
                                ptp_[:, cc * 512:(cc + 1) * 512],
                                start=(stp == 0), stop=(stp == SKT - 1),
                            )
                        if rp == 2 * SKT - 1:
                            for c in range(SQC):
                                ot = otp.tile([D + 1, 512], f32, tag="ot", name="ot")
                                nc.scalar.copy(ot[:], po[c][:])
                                nc.sync.dma_start(
                                    out[jp, :, c * 512:(c + 1) * 512], ot[:])

    nc.compile()
    return nc


def _get_nc(variant=None):
    if "nc" not in _CACHE:
        _CACHE["nc"] = _build_nc()
    return _CACHE["nc"]


def _make_in_maps(hidden_states, Wqkv_w, Wqkv_b, bias, indices, variant=None):
    hidden_states = np.asarray(hidden_states, dtype=np.float32)
    Wqkv_w = np.asarray(Wqkv_w, dtype=np.float32)
    Wqkv_b = np.asarray(Wqkv_b, dtype=np.float32)
    bias = np.asarray(bias, dtype=np.float32)
    indices = np.asarray(indices, dtype=np.int64)

    qscale = np.float32(A_EXP / math.sqrt(D))
    padded = np.zeros((TOTAL, DIM), dtype=np.float32)
    padded[indices] = hidden_states

    Wq, Wk, Wv = Wqkv_w[0:DIM], Wqkv_w[DIM:2 * DIM], Wqkv_w[2 * DIM:3 * DIM]
    bq_full = Wqkv_b[0:DIM] * qscale
    bk_full = Wqkv_b[DIM:2 * DIM]

    # path-A / path-B query column masks (within each 1024 half)
    acols = np.zeros(S, dtype=bool)
    acols[0:AW] = True
    acols[1024:1024 + AW] = True

    in_maps = []
    for cidx in range(N_CORES):
        b = cidx // 4
        h0 = (cidx % 4) * HPC
        r = slice(h0 * D, (h0 + HPC) * D)
        bias_c = np.ascontiguousarray(bias[b, h0:h0 + HPC].transpose(0, 2, 1))
        bt = np.empty((HPC, S, S), dtype=np.int16)
        bt[:, :, acols] = np.exp(bias_c[:, :, acols]).astype(np.float16).view(np.int16)
        bt[:, :, ~acols] = np.rint(
            bias_c[:, :, ~acols] * np.float32(A_EXP) + np.float32(B_OFF)
        ).astype(np.int16)
        in_maps.append({
            "hT": padded[b * S:(b + 1) * S].T.astype(np.float16),
            "wq": (Wq[r].T * qscale).astype(np.float16),
            "wk": Wk[r].T.astype(np.float16),
            "wv": Wv[r].T.astype(np.float16),
            "bq": np.ascontiguousarray(bq_full[r].reshape(HPC * D, 1)),
            "bk": np.ascontiguousarray(bk_full[r].reshape(HPC * D, 1)),
            "bias_t": bt,
        })
    return in_maps


def _assemble(results, Wqkv_b, indices):
    Wqkv_b = np.asarray(Wqkv_b, dtype=np.float32)
    indices = np.asarray(indices, dtype=np.int64)
    bv = Wqkv_b[2 * DIM:3 * DIM]
    out_full = np.empty((TOTAL, DIM), dtype=np.float32)
    for c in range(N_CORES):
        b = c // 4
        h0 = (c % 4) * HPC
        o = np.asarray(results[c]["out"], dtype=np.float32)  # (3, 65, 2048)
        for j in range(HPC):
            h = h0 + j
            att = (o[j, :D] / o[j, D]).T + bv[h * D:(h + 1) * D]
            out_full[b * S:(b + 1) * S, h * D:(h + 1) * D] = att
    return out_full[indices]


VARIANT = "v3"


def kernel(hidden_states, Wqkv_w, Wqkv_b, bias, slopes, cu_seqlens, indices,
           attn_mask, max_seqlen, **_unused):
    from concourse.bass_utils import run_bass_kernel_spmd

    nc = _get_nc()
    in_maps = _make_in_maps(hidden_states, Wqkv_w, Wqkv_b, bias, indices)
    res = run_bass_kernel_spmd(nc, in_maps, list(range(N_CORES)))
    return _assemble(res.results, Wqkv_b, indices)


# revision 5
# speedup vs baseline: 1.1269x; 1.1269x over previous
"""Trainium2 Bass kernel for BertAlibiUnpadSelfAttention.

Problem shapes (hardcoded): B=2, S=2048, H=12, D=64, DIM=768.
Reference computation:
    qkv = hidden @ Wqkv_w.T + Wqkv_b            # (4096, 2304)
    pad via indices (a permutation -> pure row shuffle)
    q,k,v = split/reshape -> (b, h, s, d)
    scores = q @ k.T / sqrt(64) + bias          # bias dense (2,12,2048,2048)
    attn = softmax(scores) @ v -> (4096, 768), unpad via indices

Sharding: 24 (batch, head) pairs -> 3 per core across 8 cores. Each core
computes its own slice of the QKV projection (disjoint columns/rows -> no
redundant FLOPs) and full attention for its 3 heads.

Device kernel layout choices (v3 - engine-balanced softmax + LAG pipeline):
  - qT/kT computed in [d, s] layout directly (lhsT = W slices, rhs = hidden^T),
    which is exactly the layout the scores matmul wants.  The Q side is
    pre-scaled by A/sqrt(D) with A = 2^10/ln2, so PSUM scores are s*A.
  - scores are computed TRANSPOSED: scoresT[sk, sq] tiles, so the softmax
    reduction (over sk) can be done by the PV matmul itself: V gets an
    appended ones-column, so PV produces [attnT ; sums] in one accumulation.
  - exp() is SPLIT across two engines by query column to balance load:
      cols 0:AW of each 1024-unit (path A): ScalarE ACTIVATE Exp
        (scale=1/A) -> fp16, then VectorE multiply by exp(bias) fp16 (2x).
      cols AW:1024 (path B): single VectorE tensor_tensor: int16(round(
        s*A + b*A + 15360 - C)) whose BITS are the fp16 Schraudolph
        approximation of exp(s+b) (+-4% sawtooth, zero mean log error with
        C=59.65; averages to ~0.1% after the PV reduction).
    The split is a pure query-column split, so each softmax row (fixed
    query, all keys) is handled by exactly one path and systematic factors
    cancel in the normalization.
  - PV matmuls are issued LAG units behind QK/evac (software pipelining),
    so the in-order PE queue never stalls waiting for the current unit's
    evacuation -> PE stays dense -> HAM clock gate stays at 8/8.
  - Final normalize (divide by sums) + transpose back to [s, d] + V-bias add
    happen on the host (tiny: 3x65x2048 per core).
"""

import math
import numpy as np

B, S, H, D = 2, 2048, 12, 64
DIM = H * D            # 768
TOTAL = B * S          # 4096
HPC = 3                # heads per core
N_CORES = 8
KT = DIM // 128        # 6 k-tiles of 128
SQC = S // 512         # 4 free-dim chunks of 512
SKT = S // 128         # 16 sk tiles of 128
VST = HPC * 65         # vp cols per st block: [h0 64 + one | h1 ... | h2 ...]

A_EXP = 1024.0 / math.log(2.0)   # fp16 Schraudolph scale, 1477.32
C_OPT = 59.65                    # zero-mean-log correction
B_OFF = 15360.0 - C_OPT          # 15*1024 - C
AW = 1024                        # path-A width per 1024-col unit (1024 = all exact exp)
LAG = 2                          # PV issue lag (units) behind QK/evac
PRE = 3                          # bias DMA prefetch depth (st tiles)

_CACHE = {}


def _build_nc():
    """Build + compile the per-core Bass module (fp16 operands, fp32 PSUM)."""
    from concourse import bacc, mybir, tile

    f32 = mybir.dt.float32
    f16 = mybir.dt.float16
    i16 = mybir.dt.int16

    nc = bacc.Bacc("TRN2", target_bir_lowering=False, debug=False)

    hT = nc.dram_tensor("hT", (DIM, S), f16, kind="ExternalInput")
    wq = nc.dram_tensor("wq", (DIM, HPC * D), f16, kind="ExternalInput")
    wk = nc.dram_tensor("wk", (DIM, HPC * D), f16, kind="ExternalInput")
    wv = nc.dram_tensor("wv", (DIM, HPC * D), f16, kind="ExternalInput")
    bq = nc.dram_tensor("bq", (HPC * D, 1), f32, kind="ExternalInput")
    bk = nc.dram_tensor("bk", (HPC * D, 1), f32, kind="ExternalInput")
    # per head [sk, sq]; in each 1024-col half: cols 0:AW fp16 exp(bias)
    # bits, cols AW:1024 int16 round(bias*A + B_OFF)
    bias_t = nc.dram_tensor("bias_t", (HPC, S, S), i16, kind="ExternalInput")
    out = nc.dram_tensor("out", (HPC, D + 1, S), f32, kind="ExternalOutput")

    EXP = mybir.ActivationFunctionType.Exp
    IDENT = mybir.ActivationFunctionType.Identity
    ADD = mybir.AluOpType.add

    with tile.TileContext(nc) as tc:
        with (
            tc.tile_pool(name="const", bufs=1) as constp,
            tc.tile_pool(name="bias", bufs=2 * PRE + 2) as biasp,
            tc.tile_pool(name="pt", bufs=LAG + 4) as ptp,
            tc.tile_pool(name="ot", bufs=3) as otp,
        ):
            # ---- load persistent inputs ----
            ht = [constp.tile([128, S], f16, tag=f"ht{i}", name=f"ht{i}") for i in range(KT)]
            for i in range(KT):
                nc.sync.dma_start(ht[i][:], hT[i * 128:(i + 1) * 128, :])

            wq_sb = [constp.tile([128, HPC * D], f16, tag=f"wq{i}", name=f"wq{i}") for i in range(KT)]
            wk_sb = [constp.tile([128, HPC * D], f16, tag=f"wk{i}", name=f"wk{i}") for i in range(KT)]
            wv_sb = [constp.tile([128, HPC * D], f16, tag=f"wv{i}", name=f"wv{i}") for i in range(KT)]
            bq_sb = constp.tile([128, 1], f32, tag="bq0")
            bq_sb2 = constp.tile([64, 1], f32, tag="bq1")
            bk_sb = constp.tile([128, 1], f32, tag="bk0")
            bk_sb2 = constp.tile([64, 1], f32, tag="bk1")
            nc.scalar.dma_start(bq_sb[:], bq[0:128, :])
            nc.scalar.dma_start(bq_sb2[:], bq[128:192, :])
            nc.scalar.dma_start(bk_sb[:], bk[0:128, :])
            nc.scalar.dma_start(bk_sb2[:], bk[128:192, :])
            for i in range(KT):
                nc.scalar.dma_start(wq_sb[i][:], wq[i * 128:(i + 1) * 128, :])
                nc.scalar.dma_start(wk_sb[i][:], wk[i * 128:(i + 1) * 128, :])
                nc.scalar.dma_start(wv_sb[i][:], wv[i * 128:(i + 1) * 128, :])
            # Q/K in [d, s] layout: heads 0,1 in tile0 (partitions 0-63 /
            # 64-127), head 2 in tile1 (partitions 0-63).  Same base
            # partition for q_j and k_j so the scores matmul operands align.
            q0 = constp.tile([128, S], f16, tag="q0")
            q1 = constp.tile([64, S], f16, tag="q1")
            k0 = constp.tile([128, S], f16, tag="k0")
            k1 = constp.tile([64, S], f16, tag="k1")
            # V' blocks per st: [h0 d0..63, one, h1 d0..63, one, h2 ...];
            # the ones come from the memset and give the softmax row-sums.
            vp = constp.tile([128, SKT * VST], f16, tag="vp")
            nc.vector.memset(vp[:], 1.0)

            # ---- phase 1a: qT / kT projection (+ bias via ScalarE) ----
            with tc.tile_pool(name="psA", bufs=2, space="PSUM") as psA:
                for (dst, wsb, bsb, col0, m) in (
                    (q0, wq_sb, bq_sb, 0, 128),
                    (q1, wq_sb, bq_sb2, 128, 64),
                    (k0, wk_sb, bk_sb, 0, 128),
                    (k1, wk_sb, bk_sb2, 128, 64),
                ):
                    for c in range(SQC):
                        ps = psA.tile([m, 512], f32, tag=f"psA{m}", name=f"psA{m}")
                        for i in range(KT):
                            nc.tensor.matmul(
                                ps[:],
                                wsb[i][:, col0:col0 + m],
                                ht[i][:, c * 512:(c + 1) * 512],
                                start=(i == 0), stop=(i == KT - 1),
                            )
                        nc.scalar.activation(
                            dst[:, c * 512:(c + 1) * 512], ps[:], IDENT,
                            bias=bsb[:])

                # ---- phase 1b: V in natural [s, d] layout ----
                for st in range(SKT):
                    psv = psA.tile([128, HPC * D], f32, tag="psV", name="psV")
                    for i in range(KT):
                        nc.tensor.matmul(
                            psv[:],
                            ht[i][:, st * 128:(st + 1) * 128],
                            wv_sb[i][:],
                            start=(i == 0), stop=(i == KT - 1),
                        )
                    for j in range(HPC):
                        nc.vector.tensor_copy(
                            vp[:, st * VST + j * 65: st * VST + j * 65 + D],
                            psv[:, j * D:(j + 1) * D])

            # ---- phase 2: attention, software-pipelined over 96 units ----
            qk_slices = (  # (q_ap, k_ap) per head, matching base partitions
                (q0[0:64, :], k0[0:64, :]),
                (q0[64:128, :], k0[64:128, :]),
                (q1[:, :], k1[:, :]),
            )
            NU = HPC * SKT * 2   # 96 units of [128 sk, 1024 sq]
            bt_tiles = {}        # flat st index (j*SKT+st) -> bias tile
            pt_tiles = {}        # unit index -> pt tile
            po_tiles = {}        # head -> [po chunks]

            def issue_bias_dma(si):
                if si >= HPC * SKT:
                    return
                j, st = divmod(si, SKT)
                bt = biasp.tile([128, S], i16, tag="bt", name=f"bt{si}")
                nc.sync.dma_start(bt[:], bias_t[j, st * 128:(st + 1) * 128, :])
                bt_tiles[si] = bt

            with (
                tc.tile_pool(name="ps", bufs=2, space="PSUM") as psp,
                tc.tile_pool(name="po", bufs=4, space="PSUM") as pop,
            ):
                for si in range(PRE):
                    issue_bias_dma(si)

                for u in range(NU + LAG):
                    if u < NU:
                        j, r = divmod(u, 2 * SKT)
                        st, half = divmod(r, 2)
                        si = j * SKT + st
                        if half == 0:
                            issue_bias_dma(si + PRE)
                        qap, kap = qk_slices[j]
                        bt = bt_tiles[si]
                        ps = psp.tile([128, 1024], f32, tag="ps", name=f"ps{u}")
                        pt = ptp.tile([128, 1024], f16, tag="pt", name=f"pt{u}")
                        pt_tiles[u] = pt
                        for cc in range(2):
                            c = half * 2 + cc
                            nc.tensor.matmul(
                                ps[:, cc * 512:(cc + 1) * 512],
                                kap[:, st * 128:(st + 1) * 128],
                                qap[:, c * 512:(c + 1) * 512],
                                start=True, stop=True,
                            )
                        hb = half * 1024
                        # path A: exact exp on ScalarE, * exp(bias) on DVE
                        nc.scalar.activation(
                            pt[:, 0:AW], ps[:, 0:AW], EXP, scale=1.0 / A_EXP)
                        nc.vector.tensor_mul(
                            pt[:, 0:AW], pt[:, 0:AW],
                            bt[:, hb:hb + AW].bitcast(f16))
                        if AW < 1024:
                            # path B: Schraudolph exp via int16 convert
                            nc.vector.tensor_tensor(
                                pt[:, AW:1024].bitcast(i16), ps[:, AW:1024],
                                bt[:, hb + AW:hb + 1024], ADD)
                    up = u - LAG
                    if up >= 0:
                        jp, rp = divmod(up, 2 * SKT)
                        stp, halfp = divmod(rp, 2)
                        if rp == 0:
                            po_tiles[jp] = [
                                pop.tile([D + 1, 512], f32, tag="po",
                                         name=f"po{jp}_{_c}")
                                for _c in range(SQC)]
                        po = po_tiles[jp]
                        ptp_ = pt_tiles.pop(up)
                        for cc in range(2):
                            c = halfp * 2 + cc
                            nc.tensor.matmul(
                                po[c][:],
                                vp[:, stp * VST + jp * 65: stp * VST + jp * 65 + D + 1],
                                ptp_[:, cc * 512:(cc + 1) * 512],
                                start=(stp == 0), stop=(stp == SKT - 1),
                            )
                        if rp == 2 * SKT - 1:
                            for c in range(SQC):
                                ot = otp.tile([D + 1, 512], f32, tag="ot", name="ot")
                                nc.scalar.copy(ot[:], po[c][:])
                                nc.sync.dma_start(
                                    out[jp, :, c * 512:(c + 1) * 512], ot[:])

    nc.compile()
    return nc


def _get_nc(variant=None):
    if "nc" not in _CACHE:
        _CACHE["nc"] = _build_nc()
    return _CACHE["nc"]


def _make_in_maps(hidden_states, Wqkv_w, Wqkv_b, bias, indices, variant=None):
    hidden_states = np.asarray(hidden_states, dtype=np.float32)
    Wqkv_w = np.asarray(Wqkv_w, dtype=np.float32)
    Wqkv_b = np.asarray(Wqkv_b, dtype=np.float32)
    bias = np.asarray(bias, dtype=np.float32)
    indices = np.asarray(indices, dtype=np.int64)

    qscale = np.float32(A_EXP / math.sqrt(D))
    padded = np.zeros((TOTAL, DIM), dtype=np.float32)
    padded[indices] = hidden_states

    Wq, Wk, Wv = Wqkv_w[0:DIM], Wqkv_w[DIM:2 * DIM], Wqkv_w[2 * DIM:3 * DIM]
    bq_full = Wqkv_b[0:DIM] * qscale
    bk_full = Wqkv_b[DIM:2 * DIM]

    # path-A / path-B query column masks (within each 1024 half)
    acols = np.zeros(S, dtype=bool)
    acols[0:AW] = True
    acols[1024:1024 + AW] = True

    in_maps = []
    for cidx in range(N_CORES):
        b = cidx // 4
        h0 = (cidx % 4) * HPC
        r = slice(h0 * D, (h0 + HPC) * D)
        bias_c = np.ascontiguousarray(bias[b, h0:h0 + HPC].transpose(0, 2, 1))
        bt = np.empty((HPC, S, S), dtype=np.int16)
        bt[:, :, acols] = np.exp(bias_c[:, :, acols]).astype(np.float16).view(np.int16)
        if not acols.all():
            bt[:, :, ~acols] = np.rint(
                bias_c[:, :, ~acols] * np.float32(A_EXP) + np.float32(B_OFF)
            ).astype(np.int16)
        in_maps.append({
            "hT": padded[b * S:(b + 1) * S].T.astype(np.float16),
            "wq": (Wq[r].T * qscale).astype(np.float16),
            "wk": Wk[r].T.astype(np.float16),
            "wv": Wv[r].T.astype(np.float16),
            "bq": np.ascontiguousarray(bq_full[r].reshape(HPC * D, 1)),
            "bk": np.ascontiguousarray(bk_full[r].reshape(HPC * D, 1)),
            "bias_t": bt,
        })
    return in_maps


def _assemble(results, Wqkv_b, indices):
    Wqkv_b = np.asarray(Wqkv_b, dtype=np.float32)
    indices = np.asarray(indices, dtype=np.int64)
    bv = Wqkv_b[2 * DIM:3 * DIM]
    out_full = np.empty((TOTAL, DIM), dtype=np.float32)
    for c in range(N_CORES):
        b = c // 4
        h0 = (c % 4) * HPC
        o = np.asarray(results[c]["out"], dtype=np.float32)  # (3, 65, 2048)
        for j in range(HPC):
            h = h0 + j
            att = (o[j, :D] / o[j, D]).T + bv[h * D:(h + 1) * D]
            out_full[b * S:(b + 1) * S, h * D:(h + 1) * D] = att
    return out_full[indices]


VARIANT = "v3"


def kernel(hidden_states, Wqkv_w, Wqkv_b, bias, slopes, cu_seqlens, indices,
           attn_mask, max_seqlen, **_unused):
    from concourse.bass_utils import run_bass_kernel_spmd

    nc = _get_nc()
    in_maps = _make_in_maps(hidden_states, Wqkv_w, Wqkv_b, bias, indices)
    res = run_bass_kernel_spmd(nc, in_maps, list(range(N_CORES)))
    return _assemble(res.results, Wqkv_b, indices)


# revision 9
# speedup vs baseline: 1.2922x; 1.1467x over previous
"""Trainium2 Bass kernel for BertAlibiUnpadSelfAttention.

Problem shapes (hardcoded): B=2, S=2048, H=12, D=64, DIM=768.
Reference computation:
    qkv = hidden @ Wqkv_w.T + Wqkv_b            # (4096, 2304)
    pad via indices (a permutation -> pure row shuffle)
    q,k,v = split/reshape -> (b, h, s, d)
    scores = q @ k.T / sqrt(64) + bias          # bias dense (2,12,2048,2048)
    attn = softmax(scores) @ v -> (4096, 768), unpad via indices

Sharding: 24 (batch, head) pairs -> 3 per core across 8 cores. Each core
computes its own slice of the QKV projection (disjoint columns/rows -> no
redundant FLOPs) and full attention for its 3 heads.

Device kernel layout choices (v5 - paired QK via PE row tiling):
  - qT/kT computed in [d, s] layout directly (lhsT = W slices, rhs = hidden^T),
    which is exactly the layout the scores matmul wants.
  - scores are computed TRANSPOSED: scoresT[sk, sq] tiles, so the softmax
    reduction (over sk) can be done by the PV matmul itself: V gets an
    appended ones-column, so PV produces [attnT ; sums] in one accumulation.
  - QK matmuls have K=64 (head dim) so they only use half the PE array's
    contraction rows.  The kernel packs TWO K=64 matmuls into the array at
    once via 64x128 row tiling (tile_position (0,0) and (64,0)): heads 0/1
    live on SBUF partitions 0-63 / 64-127 of the same q/k tiles and execute
    their QK matmuls CONCURRENTLY; head 2's q1/k1 are duplicated onto
    partitions 64-127 so two consecutive sk-tiles pair the same way.
    This halves QK PE cycles - the dominant lever because the PE spends
    most of the kernel power-throttled at 1.2 GHz (HAM K=4/8), where
    wall-clock ~ total PE cycles.
  - Each paired QK writes one [128, 1024] PSUM tile (two 512-col banks),
    so ONE ScalarE ACTIVATE Exp and ONE VectorE multiply by exp(bias)
    (shipped pre-interleaved from the host) evacuate both heads at once.
  - PV matmuls are issued LAG iterations behind QK/evac and grouped two
    iterations at a time, keeping the PE queue dense and minimizing
    64x128 <-> 128x128 tiling-mode switches.
  - Final normalize (divide by sums) + transpose back to [s, d] + V-bias add
    happen on the host (tiny: 3x65x2048 per core).
"""

import math
import numpy as np

B, S, H, D = 2, 2048, 12, 64
DIM = H * D            # 768
TOTAL = B * S          # 4096
HPC = 3                # heads per core
N_CORES = 8
KT = DIM // 128        # 6 k-tiles of 128
SQC = S // 512         # 4 free-dim chunks of 512
SKT = S // 128         # 16 sk tiles of 128
VST = HPC * 65         # vp cols per st block: [h0 64 + one | h1 ... | h2 ...]

A_EXP = 1024.0 / math.log(2.0)   # q-side pre-scale (matches exp affine)
LAG = 2                          # PV issue lag (iterations) behind QK/evac
PRE = 3                          # bias DMA prefetch depth (tiles)

_CACHE = {}


def _build_nc():
    """Build + compile the per-core Bass module (fp16 operands, fp32 PSUM)."""
    from concourse import bacc, mybir, tile

    f32 = mybir.dt.float32
    f16 = mybir.dt.float16
    i16 = mybir.dt.int16

    nc = bacc.Bacc("TRN2", target_bir_lowering=False, debug=False)

    hT = nc.dram_tensor("hT", (DIM, S), f16, kind="ExternalInput")
    wq = nc.dram_tensor("wq", (DIM, HPC * D), f16, kind="ExternalInput")
    wk = nc.dram_tensor("wk", (DIM, HPC * D), f16, kind="ExternalInput")
    wv = nc.dram_tensor("wv", (DIM, HPC * D), f16, kind="ExternalInput")
    bq = nc.dram_tensor("bq", (HPC * D, 1), f32, kind="ExternalInput")
    bk = nc.dram_tensor("bk", (HPC * D, 1), f32, kind="ExternalInput")
    # exp(bias) as fp16 bits, pre-interleaved for the paired evacuations:
    # biasA[st*128+p, cp*2048 + c2*1024 + h*512 + x] = expb[h, st*128+p,
    #   cp*1024 + c2*512 + x] for heads h in {0,1}
    # biasB[stp*128+p, cp*2048 + c2*1024 + par*512 + x] = expb[2,
    #   (2*stp+par)*128 + p, cp*1024 + c2*512 + x]
    biasA = nc.dram_tensor("biasA", (S, 2 * S), i16, kind="ExternalInput")
    biasB = nc.dram_tensor("biasB", (S // 2, 2 * S), i16, kind="ExternalInput")
    out = nc.dram_tensor("out", (HPC, D + 1, S), f32, kind="ExternalOutput")

    EXP = mybir.ActivationFunctionType.Exp
    IDENT = mybir.ActivationFunctionType.Identity

    with tile.TileContext(nc) as tc:
        with (
            tc.tile_pool(name="const", bufs=1) as constp,
            tc.tile_pool(name="bias", bufs=PRE + 2) as biasp,
            tc.tile_pool(name="pt", bufs=LAG + 4) as ptp,
            tc.tile_pool(name="ot", bufs=4) as otp,
        ):
            # ---- load persistent inputs ----
            ht = [constp.tile([128, S], f16, tag=f"ht{i}", name=f"ht{i}") for i in range(KT)]
            for i in range(KT):
                nc.sync.dma_start(ht[i][:], hT[i * 128:(i + 1) * 128, :])

            wq_sb = [constp.tile([128, HPC * D], f16, tag=f"wq{i}", name=f"wq{i}") for i in range(KT)]
            wk_sb = [constp.tile([128, HPC * D], f16, tag=f"wk{i}", name=f"wk{i}") for i in range(KT)]
            wv_sb = [constp.tile([128, HPC * D], f16, tag=f"wv{i}", name=f"wv{i}") for i in range(KT)]
            bq_sb = constp.tile([128, 1], f32, tag="bq0")
            bq_sb2 = constp.tile([64, 1], f32, tag="bq1")
            bk_sb = constp.tile([128, 1], f32, tag="bk0")
            bk_sb2 = constp.tile([64, 1], f32, tag="bk1")
            nc.scalar.dma_start(bq_sb[:], bq[0:128, :])
            nc.scalar.dma_start(bq_sb2[:], bq[128:192, :])
            nc.scalar.dma_start(bk_sb[:], bk[0:128, :])
            nc.scalar.dma_start(bk_sb2[:], bk[128:192, :])
            for i in range(KT):
                nc.scalar.dma_start(wq_sb[i][:], wq[i * 128:(i + 1) * 128, :])
                nc.scalar.dma_start(wk_sb[i][:], wk[i * 128:(i + 1) * 128, :])
                nc.scalar.dma_start(wv_sb[i][:], wv[i * 128:(i + 1) * 128, :])
            # Q/K in [d, s] layout: heads 0,1 in tile0 (partitions 0-63 /
            # 64-127); head 2 on partitions 0-63 of q1/k1, duplicated to
            # partitions 64-127 for row-tiled pairing.
            q0 = constp.tile([128, S], f16, tag="q0")
            q1 = constp.tile([128, S], f16, tag="q1")
            k0 = constp.tile([128, S], f16, tag="k0")
            k1 = constp.tile([128, S], f16, tag="k1")
            # V' blocks per st: [h0 d0..63, one, h1 d0..63, one, h2 ...];
            # the ones come from the memset and give the softmax row-sums.
            vp = constp.tile([128, SKT * VST], f16, tag="vp")
            nc.vector.memset(vp[:], 1.0)

            # ---- phase 1a: qT / kT projection (+ bias via ScalarE) ----
            with tc.tile_pool(name="psA", bufs=2, space="PSUM") as psA:
                for (dst, wsb, bsb, col0, m) in (
                    (q0, wq_sb, bq_sb, 0, 128),
                    (q1, wq_sb, bq_sb2, 128, 64),
                    (k0, wk_sb, bk_sb, 0, 128),
                    (k1, wk_sb, bk_sb2, 128, 64),
                ):
                    for c in range(SQC):
                        ps = psA.tile([m, 512], f32, tag=f"psA{m}", name=f"psA{m}")
                        for i in range(KT):
                            nc.tensor.matmul(
                                ps[:],
                                wsb[i][:, col0:col0 + m],
                                ht[i][:, c * 512:(c + 1) * 512],
                                start=(i == 0), stop=(i == KT - 1),
                            )
                        nc.scalar.activation(
                            dst[0:m, c * 512:(c + 1) * 512], ps[:], IDENT,
                            bias=bsb[:])
                # duplicate head-2 q/k onto partitions 64-127 (SBUF->SBUF)
                nc.sync.dma_start(q1[64:128, :], q1[0:64, :])
                nc.sync.dma_start(k1[64:128, :], k1[0:64, :])

                # ---- phase 1b: V in natural [s, d] layout ----
                for st in range(SKT):
                    psv = psA.tile([128, HPC * D], f32, tag="psV", name="psV")
                    for i in range(KT):
                        nc.tensor.matmul(
                            psv[:],
                            ht[i][:, st * 128:(st + 1) * 128],
                            wv_sb[i][:],
                            start=(i == 0), stop=(i == KT - 1),
                        )
                    for j in range(HPC):
                        nc.vector.tensor_copy(
                            vp[:, st * VST + j * 65: st * VST + j * 65 + D],
                            psv[:, j * D:(j + 1) * D])

            # ---- phase 2: attention, paired QK, software-pipelined ----
            def sqoff(cp, c2):
                return cp * 1024 + c2 * 512

            with (
                tc.tile_pool(name="ps", bufs=2, space="PSUM") as psp,
                tc.tile_pool(name="po", bufs=4, space="PSUM") as pop,
            ):
                # ---------- phase A: heads 0 and 1 ----------
                NIT = 2 * SKT * 2     # 64 iterations (cp, st, c2)
                bt_tiles = {}
                pt_tiles = {}
                po_t = {}

                def bias_dma_A(ti):   # ti = cp*SKT + st
                    if ti >= 2 * SKT:
                        return
                    cp, st = divmod(ti, SKT)
                    bt = biasp.tile([128, S], i16, tag="bt", name=f"btA{ti}")
                    nc.sync.dma_start(
                        bt[:], biasA[st * 128:(st + 1) * 128,
                                     cp * 2048:(cp + 1) * 2048])
                    bt_tiles[ti] = bt

                def qk_evac_A(i):
                    cp, r = divmod(i, 2 * SKT)
                    st, c2 = divmod(r, 2)
                    ti = cp * SKT + st
                    if c2 == 0:
                        bias_dma_A(ti + PRE)
                    sq = sqoff(cp, c2)
                    ps = psp.tile([128, 1024], f32, tag="ps", name=f"psA2_{i}")
                    pt = ptp.tile([128, 1024], f16, tag="pt", name=f"ptA{i}")
                    pt_tiles[i] = pt
                    nc.tensor.matmul(
                        ps[:, 0:512], k0[0:64, st * 128:(st + 1) * 128],
                        q0[0:64, sq:sq + 512], start=True, stop=True)
                    nc.tensor.matmul(
                        ps[:, 512:1024], k0[64:128, st * 128:(st + 1) * 128],
                        q0[64:128, sq:sq + 512], start=True, stop=True)
                    bt = bt_tiles[ti]
                    nc.scalar.activation(pt[:], ps[:], EXP, scale=1.0 / A_EXP)
                    nc.vector.tensor_mul(
                        pt[:], pt[:],
                        bt[:, c2 * 1024:(c2 + 1) * 1024].bitcast(f16))

                def pv_A(i):
                    cp, r = divmod(i, 2 * SKT)
                    st, c2 = divmod(r, 2)
                    key = (cp, c2)
                    if st == 0:
                        po_t[key] = [
                            pop.tile([D + 1, 512], f32, tag="po",
                                     name=f"poA{cp}_{c2}_{h}")
                            for h in range(2)]
                    po = po_t[key]
                    pt = pt_tiles.pop(i)
                    for h in range(2):
                        nc.tensor.matmul(
                            po[h][:],
                            vp[:, st * VST + h * 65: st * VST + h * 65 + D + 1],
                            pt[:, h * 512:(h + 1) * 512],
                            start=(st == 0), stop=(st == SKT - 1))
                    if st == SKT - 1:
                        for h in range(2):
                            ot = otp.tile([D + 1, 512], f32, tag="ot", name="ot")
                            nc.scalar.copy(ot[:], po[h][:])
                            nc.sync.dma_start(
                                out[h, :, sqoff(cp, c2):sqoff(cp, c2) + 512],
                                ot[:])

                for ti in range(PRE):
                    bias_dma_A(ti)
                for i0 in range(0, NIT + LAG, 2):
                    for di in range(2):
                        if i0 + di < NIT:
                            qk_evac_A(i0 + di)
                    for di in range(2):
                        ip = i0 + di - LAG
                        if 0 <= ip < NIT:
                            pv_A(ip)

                # ---------- phase B: head 2 (paired with itself) ----------
                NIT2 = 2 * (SKT // 2) * 2   # 32 iterations (cp, stp, c2)
                bt2_tiles = {}
                pt2_tiles = {}
                po2_t = {}

                def bias_dma_B(ti):   # ti = cp*(SKT//2) + stp
                    if ti >= SKT:
                        return
                    cp, stp = divmod(ti, SKT // 2)
                    bt = biasp.tile([128, S], i16, tag="bt", name=f"btB{ti}")
                    nc.sync.dma_start(
                        bt[:], biasB[stp * 128:(stp + 1) * 128,
                                     cp * 2048:(cp + 1) * 2048])
                    bt2_tiles[ti] = bt

                def qk_evac_B(i):
                    cp, r = divmod(i, SKT)
                    stp, c2 = divmod(r, 2)
                    ti = cp * (SKT // 2) + stp
                    if c2 == 0:
                        bias_dma_B(ti + PRE)
                    sq = sqoff(cp, c2)
                    st0, st1 = 2 * stp, 2 * stp + 1
                    ps = psp.tile([128, 1024], f32, tag="ps", name=f"psB2_{i}")
                    pt = ptp.tile([128, 1024], f16, tag="pt", name=f"ptB{i}")
                    pt2_tiles[i] = pt
                    nc.tensor.matmul(
                        ps[:, 0:512], k1[0:64, st0 * 128:(st0 + 1) * 128],
                        q1[0:64, sq:sq + 512], start=True, stop=True)
                    nc.tensor.matmul(
                        ps[:, 512:1024], k1[64:128, st1 * 128:(st1 + 1) * 128],
                        q1[64:128, sq:sq + 512], start=True, stop=True)
                    bt = bt2_tiles[ti]
                    nc.scalar.activation(pt[:], ps[:], EXP, scale=1.0 / A_EXP)
                    nc.vector.tensor_mul(
                        pt[:], pt[:],
                        bt[:, c2 * 1024:(c2 + 1) * 1024].bitcast(f16))

                def pv_B(i):
                    cp, r = divmod(i, SKT)
                    stp, c2 = divmod(r, 2)
                    key = (cp, c2)
                    if stp == 0:
                        po2_t[key] = pop.tile(
                            [D + 1, 512], f32, tag="po", name=f"poB{cp}_{c2}")
                    po = po2_t[key]
                    pt = pt2_tiles.pop(i)
                    for par in range(2):
                        st = 2 * stp + par
                        nc.tensor.matmul(
                            po[:],
                            vp[:, st * VST + 2 * 65: st * VST + 2 * 65 + D + 1],
                            pt[:, par * 512:(par + 1) * 512],
                            start=(stp == 0 and par == 0),
                            stop=(stp == SKT // 2 - 1 and par == 1))
                    if stp == SKT // 2 - 1:
                        ot = otp.tile([D + 1, 512], f32, tag="ot", name="ot")
                        nc.scalar.copy(ot[:], po[:])
                        nc.sync.dma_start(
                            out[2, :, sqoff(cp, c2):sqoff(cp, c2) + 512],
                            ot[:])

                for ti in range(PRE):
                    bias_dma_B(ti)
                for i0 in range(0, NIT2 + LAG, 2):
                    for di in range(2):
                        if i0 + di < NIT2:
                            qk_evac_B(i0 + di)
                    for di in range(2):
                        ip = i0 + di - LAG
                        if 0 <= ip < NIT2:
                            pv_B(ip)

    nc.compile()
    return nc


def _get_nc(variant=None):
    if "nc" not in _CACHE:
        _CACHE["nc"] = _build_nc()
    return _CACHE["nc"]


def _make_in_maps(hidden_states, Wqkv_w, Wqkv_b, bias, indices, variant=None):
    hidden_states = np.asarray(hidden_states, dtype=np.float32)
    Wqkv_w = np.asarray(Wqkv_w, dtype=np.float32)
    Wqkv_b = np.asarray(Wqkv_b, dtype=np.float32)
    bias = np.asarray(bias, dtype=np.float32)
    indices = np.asarray(indices, dtype=np.int64)

    qscale = np.float32(A_EXP / math.sqrt(D))
    padded = np.zeros((TOTAL, DIM), dtype=np.float32)
    padded[indices] = hidden_states

    Wq, Wk, Wv = Wqkv_w[0:DIM], Wqkv_w[DIM:2 * DIM], Wqkv_w[2 * DIM:3 * DIM]
    bq_full = Wqkv_b[0:DIM] * qscale
    bk_full = Wqkv_b[DIM:2 * DIM]

    in_maps = []
    for cidx in range(N_CORES):
        b = cidx // 4
        h0 = (cidx % 4) * HPC
        r = slice(h0 * D, (h0 + HPC) * D)
        bias_c = np.ascontiguousarray(bias[b, h0:h0 + HPC].transpose(0, 2, 1))
        expb = np.exp(bias_c).astype(np.float16).view(np.int16)  # [3, sk, sq]
        # biasA: [sk 2048, (cp 2, c2 2, h 2, x 512)]
        bA = expb[0:2].reshape(2, S, 2, 2, 512).transpose(1, 2, 3, 0, 4)
        bA = np.ascontiguousarray(bA.reshape(S, 2 * S))
        # biasB: [stp*128+p, (cp 2, c2 2, par 2, x 512)]
        bB = expb[2].reshape(8, 2, 128, 2, 2, 512).transpose(0, 2, 3, 4, 1, 5)
        bB = np.ascontiguousarray(bB.reshape(S // 2, 2 * S))
        in_maps.append({
            "hT": padded[b * S:(b + 1) * S].T.astype(np.float16),
            "wq": (Wq[r].T * qscale).astype(np.float16),
            "wk": Wk[r].T.astype(np.float16),
            "wv": Wv[r].T.astype(np.float16),
            "bq": np.ascontiguousarray(bq_full[r].reshape(HPC * D, 1)),
            "bk": np.ascontiguousarray(bk_full[r].reshape(HPC * D, 1)),
            "biasA": bA,
            "biasB": bB,
        })
    return in_maps


def _assemble(results, Wqkv_b, indices):
    Wqkv_b = np.asarray(Wqkv_b, dtype=np.float32)
    indices = np.asarray(indices, dtype=np.int64)
    bv = Wqkv_b[2 * DIM:3 * DIM]
    out_full = np.empty((TOTAL, DIM), dtype=np.float32)
    for c in range(N_CORES):
        b = c // 4
        h0 = (c % 4) * HPC
        o = np.asarray(results[c]["out"], dtype=np.float32)  # (3, 65, 2048)
        for j in range(HPC):
            h = h0 + j
            att = (o[j, :D] / o[j, D]).T + bv[h * D:(h + 1) * D]
            out_full[b * S:(b + 1) * S, h * D:(h + 1) * D] = att
    return out_full[indices]


VARIANT = "v5"


def kernel(hidden_states, Wqkv_w, Wqkv_b, bias, slopes, cu_seqlens, indices,
           attn_mask, max_seqlen, **_unused):
    from concourse.bass_utils import run_bass_kernel_spmd

    nc = _get_nc()
    in_maps = _make_in_maps(hidden_states, Wqkv_w, Wqkv_b, bias, indices)
    res = run_bass_kernel_spmd(nc, in_maps, list(range(N_CORES)))
    return _assemble(res.results, Wqkv_b, indices)


# revision 12
# speedup vs baseline: 1.4654x; 1.1341x over previous
"""Trainium2 Bass kernel for BertAlibiUnpadSelfAttention.

Problem shapes (hardcoded): B=2, S=2048, H=12, D=64, DIM=768.
Reference computation:
    qkv = hidden @ Wqkv_w.T + Wqkv_b            # (4096, 2304)
    pad via indices (a permutation -> pure row shuffle)
    q,k,v = split/reshape -> (b, h, s, d)
    scores = q @ k.T / sqrt(64) + bias          # bias dense (2,12,2048,2048)
    attn = softmax(scores) @ v -> (4096, 768), unpad via indices

Sharding: 24 (batch, head) pairs -> 3 per core across 8 cores. Each core
computes its own slice of the QKV projection (disjoint columns/rows -> no
redundant FLOPs) and full attention for its 3 heads.

Device kernel layout choices (v5 - paired QK via PE row tiling):
  - qT/kT computed in [d, s] layout directly (lhsT = W slices, rhs = hidden^T),
    which is exactly the layout the scores matmul wants.
  - scores are computed TRANSPOSED: scoresT[sk, sq] tiles, so the softmax
    reduction (over sk) can be done by the PV matmul itself: V gets an
    appended ones-column, so PV produces [attnT ; sums] in one accumulation.
  - QK matmuls have K=64 (head dim) so they only use half the PE array's
    contraction rows.  The kernel packs TWO K=64 matmuls into the array at
    once via 64x128 row tiling (tile_position (0,0) and (64,0)): heads 0/1
    live on SBUF partitions 0-63 / 64-127 of the same q/k tiles and execute
    their QK matmuls CONCURRENTLY; head 2's q1/k1 are duplicated onto
    partitions 64-127 so two consecutive sk-tiles pair the same way.
    This halves QK PE cycles - the dominant lever because the PE spends
    most of the kernel power-throttled at 1.2 GHz (HAM K=4/8), where
    wall-clock ~ total PE cycles.
  - Each paired QK writes one [128, 1024] PSUM tile (two 512-col banks),
    so ONE ScalarE ACTIVATE Exp and ONE VectorE multiply by exp(bias)
    (shipped pre-interleaved from the host) evacuate both heads at once.
  - PV matmuls are issued LAG iterations behind QK/evac and grouped two
    iterations at a time, keeping the PE queue dense and minimizing
    64x128 <-> 128x128 tiling-mode switches.
  - Final normalize (divide by sums) + transpose back to [s, d] + V-bias add
    happen on the host (tiny: 3x65x2048 per core).
"""

import math
import numpy as np

B, S, H, D = 2, 2048, 12, 64
DIM = H * D            # 768
TOTAL = B * S          # 4096
HPC = 3                # heads per core
N_CORES = 8
KT = DIM // 128        # 6 k-tiles of 128
SQC = S // 512         # 4 free-dim chunks of 512
SKT = S // 128         # 16 sk tiles of 128
VST = HPC * 65         # vp cols per st block: [h0 64 + one | h1 ... | h2 ...]

A_EXP = 1024.0 / math.log(2.0)   # q-side pre-scale (matches exp affine)
LAG = 2                          # PV issue lag (iterations) behind QK/evac
PRE = 3                          # bias DMA prefetch depth (tiles)

_CACHE = {}


def _build_nc():
    """Build + compile the per-core Bass module (fp16 operands, fp32 PSUM)."""
    from concourse import bacc, mybir, tile

    f32 = mybir.dt.float32
    f16 = mybir.dt.float16
    i16 = mybir.dt.int16

    nc = bacc.Bacc("TRN2", target_bir_lowering=False, debug=False)

    hT = nc.dram_tensor("hT", (DIM, S), f16, kind="ExternalInput")
    # weights packed per k-tile side by side: [p, i*192 + j] = W.T[i*128+p, j]
    wq = nc.dram_tensor("wq", (128, KT * HPC * D), f16, kind="ExternalInput")
    wk = nc.dram_tensor("wk", (128, KT * HPC * D), f16, kind="ExternalInput")
    wv = nc.dram_tensor("wv", (128, KT * HPC * D), f16, kind="ExternalInput")
    # projection bias vectors: cols = [bq 0:128, bq 128:192, bk 0:128, bk 128:192]
    bqk = nc.dram_tensor("bqk", (128, 4), f32, kind="ExternalInput")
    # exp(bias) as fp16 bits, pre-interleaved for the paired evacuations:
    # biasA[st*128+p, cp*2048 + c2*1024 + h*512 + x] = expb[h, st*128+p,
    #   cp*1024 + c2*512 + x] for heads h in {0,1}
    # biasB[stp*128+p, cp*2048 + c2*1024 + par*512 + x] = expb[2,
    #   (2*stp+par)*128 + p, cp*1024 + c2*512 + x]
    biasA = nc.dram_tensor("biasA", (S, 2 * S), i16, kind="ExternalInput")
    biasB = nc.dram_tensor("biasB", (S // 2, 2 * S), i16, kind="ExternalInput")
    out = nc.dram_tensor("out", (HPC, D + 1, S), f32, kind="ExternalOutput")

    EXP = mybir.ActivationFunctionType.Exp
    IDENT = mybir.ActivationFunctionType.Identity

    with tile.TileContext(nc) as tc:
        with (
            tc.tile_pool(name="const", bufs=1) as constp,
            tc.tile_pool(name="bias", bufs=PRE + 2) as biasp,
            tc.tile_pool(name="pt", bufs=LAG + 4) as ptp,
            tc.tile_pool(name="ot", bufs=4) as otp,
        ):
            # ---- load persistent inputs ----
            ht = [constp.tile([128, S], f16, tag=f"ht{i}", name=f"ht{i}") for i in range(KT)]
            for i in range(KT):
                eng = nc.sync if i % 2 == 0 else nc.scalar
                eng.dma_start(ht[i][:], hT[i * 128:(i + 1) * 128, :])

            WCOL = HPC * D
            wq_all = constp.tile([128, KT * WCOL], f16, tag="wqa")
            wk_all = constp.tile([128, KT * WCOL], f16, tag="wka")
            wv_all = constp.tile([128, KT * WCOL], f16, tag="wva")
            bqk_sb = constp.tile([128, 4], f32, tag="bqk")
            nc.scalar.dma_start(bqk_sb[:], bqk[:, :])
            nc.scalar.dma_start(wq_all[:], wq[:, :])
            nc.scalar.dma_start(wk_all[:], wk[:, :])
            nc.scalar.dma_start(wv_all[:], wv[:, :])
            wq_sb = [wq_all[:, i * WCOL:(i + 1) * WCOL] for i in range(KT)]
            wk_sb = [wk_all[:, i * WCOL:(i + 1) * WCOL] for i in range(KT)]
            wv_sb = [wv_all[:, i * WCOL:(i + 1) * WCOL] for i in range(KT)]
            bq_sb = bqk_sb[:, 0:1]
            bq_sb2 = bqk_sb[0:64, 1:2]
            bk_sb = bqk_sb[:, 2:3]
            bk_sb2 = bqk_sb[0:64, 3:4]
            # Q/K in [d, s] layout: heads 0,1 in tile0 (partitions 0-63 /
            # 64-127); head 2 on partitions 0-63 of q1/k1, duplicated to
            # partitions 64-127 for row-tiled pairing.
            q0 = constp.tile([128, S], f16, tag="q0")
            q1 = constp.tile([128, S], f16, tag="q1")
            k0 = constp.tile([128, S], f16, tag="k0")
            k1 = constp.tile([128, S], f16, tag="k1")
            # V' blocks per st: [h0 d0..63, one, h1 d0..63, one, h2 ...];
            # the ones come from the memset and give the softmax row-sums.
            vp = constp.tile([128, SKT * VST], f16, tag="vp")
            nc.vector.memset(vp[:], 1.0)

            # ---- phase 1a: qT / kT projection (+ bias via ScalarE) ----
            with tc.tile_pool(name="psA", bufs=2, space="PSUM") as psA:
                for (dst, wsb, bsb, col0, m) in (
                    (q0, wq_sb, bq_sb, 0, 128),
                    (q1, wq_sb, bq_sb2, 128, 64),
                    (k0, wk_sb, bk_sb, 0, 128),
                    (k1, wk_sb, bk_sb2, 128, 64),
                ):
                    for c in range(SQC):
                        ps = psA.tile([m, 512], f32, tag=f"psA{m}", name=f"psA{m}")
                        for i in range(KT):
                            nc.tensor.matmul(
                                ps[:],
                                wsb[i][:, col0:col0 + m],
                                ht[i][:, c * 512:(c + 1) * 512],
                                start=(i == 0), stop=(i == KT - 1),
                            )
                        nc.vector.tensor_scalar_add(
                            dst[0:m, c * 512:(c + 1) * 512], ps[:], bsb)
                # duplicate head-2 q/k onto partitions 64-127 (SBUF->SBUF)
                nc.sync.dma_start(q1[64:128, :], q1[0:64, :])
                nc.sync.dma_start(k1[64:128, :], k1[0:64, :])

            # ---- phase 2: attention, paired QK, software-pipelined ----
            # V projection is interleaved into phase A's idle PE slots.
            def sqoff(cp, c2):
                return cp * 1024 + c2 * 512

            with (
                tc.tile_pool(name="ps", bufs=2, space="PSUM") as psp,
                tc.tile_pool(name="po", bufs=2, space="PSUM") as pop,
                tc.tile_pool(name="psV", bufs=2, space="PSUM") as psVp,
            ):
                # ---------- phase A: heads 0 and 1 ----------
                NIT = 2 * 2 * SKT     # 64 iterations (cp, c2, st)
                bt_tiles = {}
                pt_tiles = {}
                po_t = {}

                def bias_dma_A(i):
                    if i >= NIT:
                        return
                    cpc2, st = divmod(i, SKT)
                    cp, c2 = divmod(cpc2, 2)
                    bt = biasp.tile([128, 1024], i16, tag="bt", name=f"btA{i}")
                    col = cp * 2048 + c2 * 1024
                    nc.sync.dma_start(
                        bt[:], biasA[st * 128:(st + 1) * 128, col:col + 1024])
                    bt_tiles[i] = bt

                def qk_evac_A(i):
                    cpc2, st = divmod(i, SKT)
                    cp, c2 = divmod(cpc2, 2)
                    bias_dma_A(i + PRE)
                    sq = sqoff(cp, c2)
                    ps = psp.tile([128, 1024], f32, tag="ps", name=f"psA2_{i}")
                    pt = ptp.tile([128, 1024], f16, tag="pt", name=f"ptA{i}")
                    pt_tiles[i] = pt
                    nc.tensor.matmul(
                        ps[:, 0:512], k0[0:64, st * 128:(st + 1) * 128],
                        q0[0:64, sq:sq + 512], start=True, stop=True)
                    nc.tensor.matmul(
                        ps[:, 512:1024], k0[64:128, st * 128:(st + 1) * 128],
                        q0[64:128, sq:sq + 512], start=True, stop=True)
                    nc.scalar.activation(pt[:], ps[:], EXP, scale=1.0 / A_EXP)
                    nc.vector.tensor_mul(
                        pt[:], pt[:], bt_tiles.pop(i)[:].bitcast(f16))

                def pv_A(i):
                    cpc2, st = divmod(i, SKT)
                    cp, c2 = divmod(cpc2, 2)
                    if st == 0:
                        po_t[cpc2] = [
                            pop.tile([D + 1, 512], f32, tag="po",
                                     name=f"poA{cpc2}_{h}")
                            for h in range(2)]
                    po = po_t[cpc2]
                    pt = pt_tiles.pop(i)
                    for h in range(2):
                        nc.tensor.matmul(
                            po[h][:],
                            vp[:, st * VST + h * 65: st * VST + h * 65 + D + 1],
                            pt[:, h * 512:(h + 1) * 512],
                            start=(st == 0), stop=(st == SKT - 1))
                    if st == SKT - 1:
                        for h in range(2):
                            ot = otp.tile([D + 1, 512], f32, tag="ot", name="ot")
                            nc.vector.tensor_copy(ot[:], po[h][:])
                            nc.sync.dma_start(
                                out[h, :, sqoff(cp, c2):sqoff(cp, c2) + 512],
                                ot[:])

                def v_proj(st):
                    # one V-projection packet, slotted into PE idle time
                    psv = psVp.tile([128, HPC * D], f32, tag="psV", name="psV")
                    for i in range(KT):
                        nc.tensor.matmul(
                            psv[:],
                            ht[i][:, st * 128:(st + 1) * 128],
                            wv_sb[i],
                            start=(i == 0), stop=(i == KT - 1),
                        )
                    for j in range(HPC):
                        nc.vector.tensor_copy(
                            vp[:, st * VST + j * 65: st * VST + j * 65 + D],
                            psv[:, j * D:(j + 1) * D])

                for i in range(PRE):
                    bias_dma_A(i)
                for i0 in range(0, NIT + LAG, 2):
                    g = i0 // 2
                    for di in range(2):
                        if i0 + di < NIT:
                            qk_evac_A(i0 + di)
                    for vs in (2 * g, 2 * g + 1):
                        if vs < SKT:
                            v_proj(vs)
                    for di in range(2):
                        ip = i0 + di - LAG
                        if 0 <= ip < NIT:
                            pv_A(ip)

                # ---------- phase B: head 2 (paired with itself) ----------
                NIT2 = 2 * 2 * (SKT // 2)   # 32 iterations (cp, c2, stp)
                bt2_tiles = {}
                pt2_tiles = {}
                po2_t = {}

                def bias_dma_B(i):
                    if i >= NIT2:
                        return
                    cpc2, stp = divmod(i, SKT // 2)
                    cp, c2 = divmod(cpc2, 2)
                    bt = biasp.tile([128, 1024], i16, tag="bt", name=f"btB{i}")
                    col = cp * 2048 + c2 * 1024
                    nc.sync.dma_start(
                        bt[:], biasB[stp * 128:(stp + 1) * 128, col:col + 1024])
                    bt2_tiles[i] = bt

                def qk_evac_B(i):
                    cpc2, stp = divmod(i, SKT // 2)
                    cp, c2 = divmod(cpc2, 2)
                    bias_dma_B(i + PRE)
                    sq = sqoff(cp, c2)
                    st0, st1 = 2 * stp, 2 * stp + 1
                    ps = psp.tile([128, 1024], f32, tag="ps", name=f"psB2_{i}")
                    pt = ptp.tile([128, 1024], f16, tag="pt", name=f"ptB{i}")
                    pt2_tiles[i] = pt
                    nc.tensor.matmul(
                        ps[:, 0:512], k1[0:64, st0 * 128:(st0 + 1) * 128],
                        q1[0:64, sq:sq + 512], start=True, stop=True)
                    nc.tensor.matmul(
                        ps[:, 512:1024], k1[64:128, st1 * 128:(st1 + 1) * 128],
                        q1[64:128, sq:sq + 512], start=True, stop=True)
                    nc.scalar.activation(pt[:], ps[:], EXP, scale=1.0 / A_EXP)
                    nc.vector.tensor_mul(
                        pt[:], pt[:], bt2_tiles.pop(i)[:].bitcast(f16))

                def pv_B(i):
                    cpc2, stp = divmod(i, SKT // 2)
                    cp, c2 = divmod(cpc2, 2)
                    if stp == 0:
                        po2_t[cpc2] = pop.tile(
                            [D + 1, 512], f32, tag="po", name=f"poB{cpc2}")
                    po = po2_t[cpc2]
                    pt = pt2_tiles.pop(i)
                    for par in range(2):
                        st = 2 * stp + par
                        nc.tensor.matmul(
                            po[:],
                            vp[:, st * VST + 2 * 65: st * VST + 2 * 65 + D + 1],
                            pt[:, par * 512:(par + 1) * 512],
                            start=(stp == 0 and par == 0),
                            stop=(stp == SKT // 2 - 1 and par == 1))
                    if stp == SKT // 2 - 1:
                        ot = otp.tile([D + 1, 512], f32, tag="ot", name="ot")
                        nc.vector.tensor_copy(ot[:], po[:])
                        nc.scalar.dma_start(
                            out[2, :, sqoff(cp, c2):sqoff(cp, c2) + 512],
                            ot[:])

                for i in range(PRE):
                    bias_dma_B(i)
                for i0 in range(0, NIT2 + LAG, 2):
                    for di in range(2):
                        if i0 + di < NIT2:
                            qk_evac_B(i0 + di)
                    for di in range(2):
                        ip = i0 + di - LAG
                        if 0 <= ip < NIT2:
                            pv_B(ip)

    nc.compile()
    return nc


def _get_nc(variant=None):
    if "nc" not in _CACHE:
        _CACHE["nc"] = _build_nc()
    return _CACHE["nc"]


def _make_in_maps(hidden_states, Wqkv_w, Wqkv_b, bias, indices, variant=None):
    hidden_states = np.asarray(hidden_states, dtype=np.float32)
    Wqkv_w = np.asarray(Wqkv_w, dtype=np.float32)
    Wqkv_b = np.asarray(Wqkv_b, dtype=np.float32)
    bias = np.asarray(bias, dtype=np.float32)
    indices = np.asarray(indices, dtype=np.int64)

    qscale = np.float32(A_EXP / math.sqrt(D))
    padded = np.zeros((TOTAL, DIM), dtype=np.float32)
    padded[indices] = hidden_states

    Wq, Wk, Wv = Wqkv_w[0:DIM], Wqkv_w[DIM:2 * DIM], Wqkv_w[2 * DIM:3 * DIM]
    bq_full = Wqkv_b[0:DIM] * qscale
    bk_full = Wqkv_b[DIM:2 * DIM]

    def pack_w(WT):  # [768, 192] -> [128, 6*192]
        return np.ascontiguousarray(
            WT.reshape(KT, 128, HPC * D).transpose(1, 0, 2).reshape(
                128, KT * HPC * D))

    in_maps = []
    for cidx in range(N_CORES):
        b = cidx // 4
        h0 = (cidx % 4) * HPC
        r = slice(h0 * D, (h0 + HPC) * D)
        bias_c = np.ascontiguousarray(bias[b, h0:h0 + HPC].transpose(0, 2, 1))
        expb = np.exp(bias_c).astype(np.float16).view(np.int16)  # [3, sk, sq]
        # biasA: [sk 2048, (cp 2, c2 2, h 2, x 512)]
        bA = expb[0:2].reshape(2, S, 2, 2, 512).transpose(1, 2, 3, 0, 4)
        bA = np.ascontiguousarray(bA.reshape(S, 2 * S))
        # biasB: [stp*128+p, (cp 2, c2 2, par 2, x 512)]
        bB = expb[2].reshape(8, 2, 128, 2, 2, 512).transpose(0, 2, 3, 4, 1, 5)
        bB = np.ascontiguousarray(bB.reshape(S // 2, 2 * S))
        bqk_arr = np.zeros((128, 4), dtype=np.float32)
        bqk_arr[:, 0] = bq_full[r][0:128]
        bqk_arr[0:64, 1] = bq_full[r][128:192]
        bqk_arr[:, 2] = bk_full[r][0:128]
        bqk_arr[0:64, 3] = bk_full[r][128:192]
        in_maps.append({
            "hT": padded[b * S:(b + 1) * S].T.astype(np.float16),
            "wq": pack_w((Wq[r].T * qscale).astype(np.float16)),
            "wk": pack_w(Wk[r].T.astype(np.float16)),
            "wv": pack_w(Wv[r].T.astype(np.float16)),
            "bqk": bqk_arr,
            "biasA": bA,
            "biasB": bB,
        })
    return in_maps


def _assemble(results, Wqkv_b, indices):
    Wqkv_b = np.asarray(Wqkv_b, dtype=np.float32)
    indices = np.asarray(indices, dtype=np.int64)
    bv = Wqkv_b[2 * DIM:3 * DIM]
    out_full = np.empty((TOTAL, DIM), dtype=np.float32)
    for c in range(N_CORES):
        b = c // 4
        h0 = (c % 4) * HPC
        o = np.asarray(results[c]["out"], dtype=np.float32)  # (3, 65, 2048)
        for j in range(HPC):
            h = h0 + j
            att = (o[j, :D] / o[j, D]).T + bv[h * D:(h + 1) * D]
            out_full[b * S:(b + 1) * S, h * D:(h + 1) * D] = att
    return out_full[indices]


VARIANT = "v5"


def kernel(hidden_states, Wqkv_w, Wqkv_b, bias, slopes, cu_seqlens, indices,
           attn_mask, max_seqlen, **_unused):
    from concourse.bass_utils import run_bass_kernel_spmd

    nc = _get_nc()
    in_maps = _make_in_maps(hidden_states, Wqkv_w, Wqkv_b, bias, indices)
    res = run_bass_kernel_spmd(nc, in_maps, list(range(N_CORES)))
    return _assemble(res.results, Wqkv_b, indices)


# revision 13
# speedup vs baseline: 1.6842x; 1.1493x over previous
"""Trainium2 Bass kernel for BertAlibiUnpadSelfAttention.

Problem shapes (hardcoded): B=2, S=2048, H=12, D=64, DIM=768.
Reference computation:
    qkv = hidden @ Wqkv_w.T + Wqkv_b            # (4096, 2304)
    pad via indices (a permutation -> pure row shuffle)
    q,k,v = split/reshape -> (b, h, s, d)
    scores = q @ k.T / sqrt(64) + bias          # bias dense (2,12,2048,2048)
    attn = softmax(scores) @ v -> (4096, 768), unpad via indices

Sharding: 24 (batch, head) pairs -> 3 per core across 8 cores. Each core
computes its own slice of the QKV projection (disjoint columns/rows -> no
redundant FLOPs) and full attention for its 3 heads.

Device kernel layout choices (v5 - paired QK via PE row tiling):
  - qT/kT computed in [d, s] layout directly (lhsT = W slices, rhs = hidden^T),
    which is exactly the layout the scores matmul wants.
  - scores are computed TRANSPOSED: scoresT[sk, sq] tiles, so the softmax
    reduction (over sk) can be done by the PV matmul itself: V gets an
    appended ones-column, so PV produces [attnT ; sums] in one accumulation.
  - QK matmuls have K=64 (head dim) so they only use half the PE array's
    contraction rows.  The kernel packs TWO K=64 matmuls into the array at
    once via 64x128 row tiling (tile_position (0,0) and (64,0)): heads 0/1
    live on SBUF partitions 0-63 / 64-127 of the same q/k tiles and execute
    their QK matmuls CONCURRENTLY; head 2's q1/k1 are duplicated onto
    partitions 64-127 so two consecutive sk-tiles pair the same way.
    This halves QK PE cycles - the dominant lever because the PE spends
    most of the kernel power-throttled at 1.2 GHz (HAM K=4/8), where
    wall-clock ~ total PE cycles.
  - Each paired QK writes one [128, 1024] PSUM tile (two 512-col banks),
    so ONE ScalarE ACTIVATE Exp and ONE VectorE multiply by exp(bias)
    (shipped pre-interleaved from the host) evacuate both heads at once.
  - PV matmuls are issued LAG iterations behind QK/evac and grouped two
    iterations at a time, keeping the PE queue dense and minimizing
    64x128 <-> 128x128 tiling-mode switches.
  - Final normalize (divide by sums) + transpose back to [s, d] + V-bias add
    happen on the host (tiny: 3x65x2048 per core).
"""

import math
import numpy as np

B, S, H, D = 2, 2048, 12, 64
DIM = H * D            # 768
TOTAL = B * S          # 4096
HPC = 3                # heads per core
N_CORES = 8
KT = DIM // 128        # 6 k-tiles of 128
SQC = S // 512         # 4 free-dim chunks of 512
SKT = S // 128         # 16 sk tiles of 128
VST = HPC * 65         # vp cols per st block: [h0 64 + one | h1 ... | h2 ...]

A_EXP = 1024.0 / math.log(2.0)   # q-side pre-scale (matches exp affine)
LAG = 2                          # PV issue lag (iterations) behind QK/evac
PRE = 3                          # bias DMA prefetch depth (tiles)

_CACHE = {}


def _build_nc():
    """Build + compile the per-core Bass module (fp16 operands, fp32 PSUM)."""
    from concourse import bacc, mybir, tile

    f32 = mybir.dt.float32
    f16 = mybir.dt.float16
    i16 = mybir.dt.int16

    nc = bacc.Bacc("TRN2", target_bir_lowering=False, debug=False)

    hT = nc.dram_tensor("hT", (DIM, S), f16, kind="ExternalInput")
    # weights packed per k-tile side by side: [p, i*192 + j] = W.T[i*128+p, j]
    wq = nc.dram_tensor("wq", (128, KT * HPC * D), f16, kind="ExternalInput")
    wk = nc.dram_tensor("wk", (128, KT * HPC * D), f16, kind="ExternalInput")
    wv = nc.dram_tensor("wv", (128, KT * HPC * D), f16, kind="ExternalInput")
    # projection bias vectors: cols = [bq 0:128, bq 128:192, bk 0:128, bk 128:192]
    bqk = nc.dram_tensor("bqk", (128, 4), f32, kind="ExternalInput")
    # exp(bias) as fp16 bits, pre-interleaved for the paired evacuations:
    # biasA[st*128+p, cp*2048 + c2*1024 + h*512 + x] = expb[h, st*128+p,
    #   cp*1024 + c2*512 + x] for heads h in {0,1}
    # biasB[stp*128+p, cp*2048 + c2*1024 + par*512 + x] = expb[2,
    #   (2*stp+par)*128 + p, cp*1024 + c2*512 + x]
    biasA = nc.dram_tensor("biasA", (S, 2 * S), i16, kind="ExternalInput")
    biasB = nc.dram_tensor("biasB", (S // 2, 2 * S), i16, kind="ExternalInput")
    out = nc.dram_tensor("out", (HPC, D + 1, S), f32, kind="ExternalOutput")

    EXP = mybir.ActivationFunctionType.Exp
    IDENT = mybir.ActivationFunctionType.Identity

    with tile.TileContext(nc) as tc:
        with (
            tc.tile_pool(name="const", bufs=1) as constp,
            tc.tile_pool(name="bias", bufs=PRE + 2) as biasp,
            tc.tile_pool(name="pt", bufs=LAG + 4) as ptp,
            tc.tile_pool(name="ot", bufs=4) as otp,
        ):
            # ---- load persistent inputs ----
            ht = [constp.tile([128, S], f16, tag=f"ht{i}", name=f"ht{i}") for i in range(KT)]
            for i in range(KT):
                nc.sync.dma_start(ht[i][:], hT[i * 128:(i + 1) * 128, :])

            WCOL = HPC * D
            wq_all = constp.tile([128, KT * WCOL], f16, tag="wqa")
            wk_all = constp.tile([128, KT * WCOL], f16, tag="wka")
            wv_all = constp.tile([128, KT * WCOL], f16, tag="wva")
            bqk_sb = constp.tile([128, 4], f32, tag="bqk")
            nc.scalar.dma_start(bqk_sb[:], bqk[:, :])
            nc.scalar.dma_start(wq_all[:], wq[:, :])
            nc.scalar.dma_start(wk_all[:], wk[:, :])
            nc.scalar.dma_start(wv_all[:], wv[:, :])
            wq_sb = [wq_all[:, i * WCOL:(i + 1) * WCOL] for i in range(KT)]
            wk_sb = [wk_all[:, i * WCOL:(i + 1) * WCOL] for i in range(KT)]
            wv_sb = [wv_all[:, i * WCOL:(i + 1) * WCOL] for i in range(KT)]
            bq_sb = bqk_sb[:, 0:1]
            bq_sb2 = bqk_sb[0:64, 1:2]
            bk_sb = bqk_sb[:, 2:3]
            bk_sb2 = bqk_sb[0:64, 3:4]
            # Q/K in [d, s] layout: heads 0,1 in tile0 (partitions 0-63 /
            # 64-127); head 2 on partitions 0-63 of q1/k1, duplicated to
            # partitions 64-127 for row-tiled pairing.
            q0 = constp.tile([128, S], f16, tag="q0")
            q1 = constp.tile([128, S], f16, tag="q1")
            k0 = constp.tile([128, S], f16, tag="k0")
            k1 = constp.tile([128, S], f16, tag="k1")
            # V' blocks per st: [h0 d0..63, one, h1 d0..63, one, h2 ...];
            # the ones come from the memset and give the softmax row-sums.
            vp = constp.tile([128, SKT * VST], f16, tag="vp")
            warm = constp.tile([128, 512], f16, tag="warm")
            nc.vector.memset(warm[:], 0.5)
            nc.vector.memset(vp[:], 1.0)

            # ---- phase 1a: qT / kT projection (+ bias via ScalarE) ----
            with tc.tile_pool(name="psA", bufs=2, space="PSUM") as psA:
                # PE warmup during the input-DMA window: keeps the HAM clock
                # gate at 8/8 so projection and attention run at 2.4 GHz
                for w in range(46):
                    wps = psA.tile([128, 512], f32, tag="psA128", name=f"wu{w}")
                    nc.tensor.matmul(
                        wps[:], warm[:, 0:128], warm[:],
                        start=True, stop=True)
                for (dst, wsb, bsb, col0, m) in (
                    (q0, wq_sb, bq_sb, 0, 128),
                    (q1, wq_sb, bq_sb2, 128, 64),
                    (k0, wk_sb, bk_sb, 0, 128),
                    (k1, wk_sb, bk_sb2, 128, 64),
                ):
                    for c in range(SQC):
                        ps = psA.tile([m, 512], f32, tag=f"psA{m}", name=f"psA{m}")
                        for i in range(KT):
                            nc.tensor.matmul(
                                ps[:],
                                wsb[i][:, col0:col0 + m],
                                ht[i][:, c * 512:(c + 1) * 512],
                                start=(i == 0), stop=(i == KT - 1),
                            )
                        nc.vector.tensor_scalar_add(
                            dst[0:m, c * 512:(c + 1) * 512], ps[:], bsb)
                # duplicate head-2 q/k onto partitions 64-127 (SBUF->SBUF)
                nc.sync.dma_start(q1[64:128, :], q1[0:64, :])
                nc.sync.dma_start(k1[64:128, :], k1[0:64, :])

            # ---- phase 2: attention, paired QK, software-pipelined ----
            # V projection is interleaved into phase A's idle PE slots.
            def sqoff(cp, c2):
                return cp * 1024 + c2 * 512

            with (
                tc.tile_pool(name="ps", bufs=2, space="PSUM") as psp,
                tc.tile_pool(name="po", bufs=2, space="PSUM") as pop,
                tc.tile_pool(name="psV", bufs=2, space="PSUM") as psVp,
            ):
                # ---------- phase A: heads 0 and 1 ----------
                NIT = 2 * 2 * SKT     # 64 iterations (cp, c2, st)
                bt_tiles = {}
                pt_tiles = {}
                po_t = {}

                def bias_dma_A(i):
                    if i >= NIT:
                        return
                    cpc2, st = divmod(i, SKT)
                    cp, c2 = divmod(cpc2, 2)
                    bt = biasp.tile([128, 1024], i16, tag="bt", name=f"btA{i}")
                    col = cp * 2048 + c2 * 1024
                    nc.sync.dma_start(
                        bt[:], biasA[st * 128:(st + 1) * 128, col:col + 1024])
                    bt_tiles[i] = bt

                def qk_evac_A(i):
                    cpc2, st = divmod(i, SKT)
                    cp, c2 = divmod(cpc2, 2)
                    bias_dma_A(i + PRE)
                    sq = sqoff(cp, c2)
                    ps = psp.tile([128, 1024], f32, tag="ps", name=f"psA2_{i}")
                    pt = ptp.tile([128, 1024], f16, tag="pt", name=f"ptA{i}")
                    pt_tiles[i] = pt
                    nc.tensor.matmul(
                        ps[:, 0:512], k0[0:64, st * 128:(st + 1) * 128],
                        q0[0:64, sq:sq + 512], start=True, stop=True)
                    nc.tensor.matmul(
                        ps[:, 512:1024], k0[64:128, st * 128:(st + 1) * 128],
                        q0[64:128, sq:sq + 512], start=True, stop=True)
                    nc.scalar.activation(pt[:], ps[:], EXP, scale=1.0 / A_EXP)
                    nc.vector.tensor_mul(
                        pt[:], pt[:], bt_tiles.pop(i)[:].bitcast(f16))

                def pv_A(i):
                    cpc2, st = divmod(i, SKT)
                    cp, c2 = divmod(cpc2, 2)
                    if st == 0:
                        po_t[cpc2] = [
                            pop.tile([D + 1, 512], f32, tag="po",
                                     name=f"poA{cpc2}_{h}")
                            for h in range(2)]
                    po = po_t[cpc2]
                    pt = pt_tiles.pop(i)
                    for h in range(2):
                        nc.tensor.matmul(
                            po[h][:],
                            vp[:, st * VST + h * 65: st * VST + h * 65 + D + 1],
                            pt[:, h * 512:(h + 1) * 512],
                            start=(st == 0), stop=(st == SKT - 1))
                    if st == SKT - 1:
                        for h in range(2):
                            ot = otp.tile([D + 1, 512], f32, tag="ot", name="ot")
                            nc.vector.tensor_copy(ot[:], po[h][:])
                            nc.sync.dma_start(
                                out[h, :, sqoff(cp, c2):sqoff(cp, c2) + 512],
                                ot[:])

                def v_proj(st):
                    # one V-projection packet, slotted into PE idle time
                    psv = psVp.tile([128, HPC * D], f32, tag="psV", name="psV")
                    for i in range(KT):
                        nc.tensor.matmul(
                            psv[:],
                            ht[i][:, st * 128:(st + 1) * 128],
                            wv_sb[i],
                            start=(i == 0), stop=(i == KT - 1),
                        )
                    nc.vector.tensor_copy(
                        vp[:, st * VST: st * VST + VST].rearrange(
                            "p (j d) -> p j d", j=HPC)[:, :, 0:D],
                        psv[:].rearrange("p (j d) -> p j d", j=HPC))

                for i in range(PRE):
                    bias_dma_A(i)
                for i0 in range(0, NIT + LAG, 2):
                    g = i0 // 2
                    for di in range(2):
                        if i0 + di < NIT:
                            qk_evac_A(i0 + di)
                    for vs in (2 * g, 2 * g + 1):
                        if vs < SKT:
                            v_proj(vs)
                    for di in range(2):
                        ip = i0 + di - LAG
                        if 0 <= ip < NIT:
                            pv_A(ip)

                # ---------- phase B: head 2 (paired with itself) ----------
                NIT2 = 2 * 2 * (SKT // 2)   # 32 iterations (cp, c2, stp)
                bt2_tiles = {}
                pt2_tiles = {}
                po2_t = {}

                def bias_dma_B(i):
                    if i >= NIT2:
                        return
                    cpc2, stp = divmod(i, SKT // 2)
                    cp, c2 = divmod(cpc2, 2)
                    bt = biasp.tile([128, 1024], i16, tag="bt", name=f"btB{i}")
                    col = cp * 2048 + c2 * 1024
                    nc.sync.dma_start(
                        bt[:], biasB[stp * 128:(stp + 1) * 128, col:col + 1024])
                    bt2_tiles[i] = bt

                def qk_evac_B(i):
                    cpc2, stp = divmod(i, SKT // 2)
                    cp, c2 = divmod(cpc2, 2)
                    bias_dma_B(i + PRE)
                    sq = sqoff(cp, c2)
                    st0, st1 = 2 * stp, 2 * stp + 1
                    ps = psp.tile([128, 1024], f32, tag="ps", name=f"psB2_{i}")
                    pt = ptp.tile([128, 1024], f16, tag="pt", name=f"ptB{i}")
                    pt2_tiles[i] = pt
                    nc.tensor.matmul(
                        ps[:, 0:512], k1[0:64, st0 * 128:(st0 + 1) * 128],
                        q1[0:64, sq:sq + 512], start=True, stop=True)
                    nc.tensor.matmul(
                        ps[:, 512:1024], k1[64:128, st1 * 128:(st1 + 1) * 128],
                        q1[64:128, sq:sq + 512], start=True, stop=True)
                    nc.scalar.activation(pt[:], ps[:], EXP, scale=1.0 / A_EXP)
                    nc.vector.tensor_mul(
                        pt[:], pt[:], bt2_tiles.pop(i)[:].bitcast(f16))

                def pv_B(i):
                    cpc2, stp = divmod(i, SKT // 2)
                    cp, c2 = divmod(cpc2, 2)
                    if stp == 0:
                        po2_t[cpc2] = pop.tile(
                            [D + 1, 512], f32, tag="po", name=f"poB{cpc2}")
                    po = po2_t[cpc2]
                    pt = pt2_tiles.pop(i)
                    for par in range(2):
                        st = 2 * stp + par
                        nc.tensor.matmul(
                            po[:],
                            vp[:, st * VST + 2 * 65: st * VST + 2 * 65 + D + 1],
                            pt[:, par * 512:(par + 1) * 512],
                            start=(stp == 0 and par == 0),
                            stop=(stp == SKT // 2 - 1 and par == 1))
                    if stp == SKT // 2 - 1:
                        ot = otp.tile([D + 1, 512], f32, tag="ot", name="ot")
                        nc.vector.tensor_copy(ot[:], po[:])
                        nc.scalar.dma_start(
                            out[2, :, sqoff(cp, c2):sqoff(cp, c2) + 512],
                            ot[:])

                for i in range(PRE):
                    bias_dma_B(i)
                for i0 in range(0, NIT2 + LAG, 2):
                    for di in range(2):
                        if i0 + di < NIT2:
                            qk_evac_B(i0 + di)
                    for di in range(2):
                        ip = i0 + di - LAG
                        if 0 <= ip < NIT2:
                            pv_B(ip)

    nc.compile()
    return nc


def _get_nc(variant=None):
    if "nc" not in _CACHE:
        _CACHE["nc"] = _build_nc()
    return _CACHE["nc"]


def _make_in_maps(hidden_states, Wqkv_w, Wqkv_b, bias, indices, variant=None):
    hidden_states = np.asarray(hidden_states, dtype=np.float32)
    Wqkv_w = np.asarray(Wqkv_w, dtype=np.float32)
    Wqkv_b = np.asarray(Wqkv_b, dtype=np.float32)
    bias = np.asarray(bias, dtype=np.float32)
    indices = np.asarray(indices, dtype=np.int64)

    qscale = np.float32(A_EXP / math.sqrt(D))
    padded = np.zeros((TOTAL, DIM), dtype=np.float32)
    padded[indices] = hidden_states

    Wq, Wk, Wv = Wqkv_w[0:DIM], Wqkv_w[DIM:2 * DIM], Wqkv_w[2 * DIM:3 * DIM]
    bq_full = Wqkv_b[0:DIM] * qscale
    bk_full = Wqkv_b[DIM:2 * DIM]

    def pack_w(WT):  # [768, 192] -> [128, 6*192]
        return np.ascontiguousarray(
            WT.reshape(KT, 128, HPC * D).transpose(1, 0, 2).reshape(
                128, KT * HPC * D))

    in_maps = []
    for cidx in range(N_CORES):
        b = cidx // 4
        h0 = (cidx % 4) * HPC
        r = slice(h0 * D, (h0 + HPC) * D)
        bias_c = np.ascontiguousarray(bias[b, h0:h0 + HPC].transpose(0, 2, 1))
        expb = np.exp(bias_c).astype(np.float16).view(np.int16)  # [3, sk, sq]
        # biasA: [sk 2048, (cp 2, c2 2, h 2, x 512)]
        bA = expb[0:2].reshape(2, S, 2, 2, 512).transpose(1, 2, 3, 0, 4)
        bA = np.ascontiguousarray(bA.reshape(S, 2 * S))
        # biasB: [stp*128+p, (cp 2, c2 2, par 2, x 512)]
        bB = expb[2].reshape(8, 2, 128, 2, 2, 512).transpose(0, 2, 3, 4, 1, 5)
        bB = np.ascontiguousarray(bB.reshape(S // 2, 2 * S))
        bqk_arr = np.zeros((128, 4), dtype=np.float32)
        bqk_arr[:, 0] = bq_full[r][0:128]
        bqk_arr[0:64, 1] = bq_full[r][128:192]
        bqk_arr[:, 2] = bk_full[r][0:128]
        bqk_arr[0:64, 3] = bk_full[r][128:192]
        in_maps.append({
            "hT": padded[b * S:(b + 1) * S].T.astype(np.float16),
            "wq": pack_w((Wq[r].T * qscale).astype(np.float16)),
            "wk": pack_w(Wk[r].T.astype(np.float16)),
            "wv": pack_w(Wv[r].T.astype(np.float16)),
            "bqk": bqk_arr,
            "biasA": bA,
            "biasB": bB,
        })
    return in_maps


def _assemble(results, Wqkv_b, indices):
    Wqkv_b = np.asarray(Wqkv_b, dtype=np.float32)
    indices = np.asarray(indices, dtype=np.int64)
    bv = Wqkv_b[2 * DIM:3 * DIM]
    out_full = np.empty((TOTAL, DIM), dtype=np.float32)
    for c in range(N_CORES):
        b = c // 4
        h0 = (c % 4) * HPC
        o = np.asarray(results[c]["out"], dtype=np.float32)  # (3, 65, 2048)
        for j in range(HPC):
            h = h0 + j
            att = (o[j, :D] / o[j, D]).T + bv[h * D:(h + 1) * D]
            out_full[b * S:(b + 1) * S, h * D:(h + 1) * D] = att
    return out_full[indices]


VARIANT = "v5"


def kernel(hidden_states, Wqkv_w, Wqkv_b, bias, slopes, cu_seqlens, indices,
           attn_mask, max_seqlen, **_unused):
    from concourse.bass_utils import run_bass_kernel_spmd

    nc = _get_nc()
    in_maps = _make_in_maps(hidden_states, Wqkv_w, Wqkv_b, bias, indices)
    res = run_bass_kernel_spmd(nc, in_maps, list(range(N_CORES)))
    return _assemble(res.results, Wqkv_b, indices)


# revision 14
# speedup vs baseline: 1.7312x; 1.0279x over previous
"""Trainium2 Bass kernel for BertAlibiUnpadSelfAttention.

Problem shapes (hardcoded): B=2, S=2048, H=12, D=64, DIM=768.
Reference computation:
    qkv = hidden @ Wqkv_w.T + Wqkv_b            # (4096, 2304)
    pad via indices (a permutation -> pure row shuffle)
    q,k,v = split/reshape -> (b, h, s, d)
    scores = q @ k.T / sqrt(64) + bias          # bias dense (2,12,2048,2048)
    attn = softmax(scores) @ v -> (4096, 768), unpad via indices

Sharding: 24 (batch, head) pairs -> 3 per core across 8 cores. Each core
computes its own slice of the QKV projection (disjoint columns/rows -> no
redundant FLOPs) and full attention for its 3 heads.

Device kernel layout choices (v5 - paired QK via PE row tiling):
  - qT/kT computed in [d, s] layout directly (lhsT = W slices, rhs = hidden^T),
    which is exactly the layout the scores matmul wants.
  - scores are computed TRANSPOSED: scoresT[sk, sq] tiles, so the softmax
    reduction (over sk) can be done by the PV matmul itself: V gets an
    appended ones-column, so PV produces [attnT ; sums] in one accumulation.
  - QK matmuls have K=64 (head dim) so they only use half the PE array's
    contraction rows.  The kernel packs TWO K=64 matmuls into the array at
    once via 64x128 row tiling (tile_position (0,0) and (64,0)): heads 0/1
    live on SBUF partitions 0-63 / 64-127 of the same q/k tiles and execute
    their QK matmuls CONCURRENTLY; head 2's q1/k1 are duplicated onto
    partitions 64-127 so two consecutive sk-tiles pair the same way.
    This halves QK PE cycles - the dominant lever because the PE spends
    most of the kernel power-throttled at 1.2 GHz (HAM K=4/8), where
    wall-clock ~ total PE cycles.
  - Each paired QK writes one [128, 1024] PSUM tile (two 512-col banks),
    so ONE ScalarE ACTIVATE Exp and ONE VectorE multiply by exp(bias)
    (shipped pre-interleaved from the host) evacuate both heads at once.
  - PV matmuls are issued LAG iterations behind QK/evac and grouped two
    iterations at a time, keeping the PE queue dense and minimizing
    64x128 <-> 128x128 tiling-mode switches.
  - Final normalize (divide by sums) + transpose back to [s, d] + V-bias add
    happen on the host (tiny: 3x65x2048 per core).
"""

import math
import numpy as np

B, S, H, D = 2, 2048, 12, 64
DIM = H * D            # 768
TOTAL = B * S          # 4096
HPC = 3                # heads per core
N_CORES = 8
KT = DIM // 128        # 6 k-tiles of 128
SQC = S // 512         # 4 free-dim chunks of 512
SKT = S // 128         # 16 sk tiles of 128
VST = HPC * 65         # vp cols per st block: [h0 64 + one | h1 ... | h2 ...]

A_EXP = 1024.0 / math.log(2.0)   # q-side pre-scale (matches exp affine)
LAG = 2                          # PV issue lag (iterations) behind QK/evac
PRE = 3                          # bias DMA prefetch depth (tiles)

_CACHE = {}


def _build_nc():
    """Build + compile the per-core Bass module (fp16 operands, fp32 PSUM)."""
    from concourse import bacc, mybir, tile

    f32 = mybir.dt.float32
    f16 = mybir.dt.float16
    i16 = mybir.dt.int16

    nc = bacc.Bacc("TRN2", target_bir_lowering=False, debug=False)

    hT = nc.dram_tensor("hT", (DIM, S), f16, kind="ExternalInput")
    # weights packed per k-tile side by side: [p, i*192 + j] = W.T[i*128+p, j]
    wq = nc.dram_tensor("wq", (128, KT * HPC * D), f16, kind="ExternalInput")
    wk = nc.dram_tensor("wk", (128, KT * HPC * D), f16, kind="ExternalInput")
    wv = nc.dram_tensor("wv", (128, KT * HPC * D), f16, kind="ExternalInput")
    # projection bias vectors: cols = [bq 0:128, bq 128:192, bk 0:128, bk 128:192]
    bqk = nc.dram_tensor("bqk", (128, 4), f32, kind="ExternalInput")
    # exp(bias) as fp16 bits, pre-interleaved for the paired evacuations:
    # biasA[st*128+p, cp*2048 + c2*1024 + h*512 + x] = expb[h, st*128+p,
    #   cp*1024 + c2*512 + x] for heads h in {0,1}
    # biasB[stp*128+p, cp*2048 + c2*1024 + par*512 + x] = expb[2,
    #   (2*stp+par)*128 + p, cp*1024 + c2*512 + x]
    biasA = nc.dram_tensor("biasA", (S, 2 * S), i16, kind="ExternalInput")
    biasB = nc.dram_tensor("biasB", (S // 2, 2 * S), i16, kind="ExternalInput")
    out = nc.dram_tensor("out", (HPC, D + 1, S), f32, kind="ExternalOutput")

    EXP = mybir.ActivationFunctionType.Exp
    IDENT = mybir.ActivationFunctionType.Identity

    with tile.TileContext(nc) as tc:
        with (
            tc.tile_pool(name="const", bufs=1) as constp,
            tc.tile_pool(name="bias", bufs=PRE + 2) as biasp,
            tc.tile_pool(name="pt", bufs=LAG + 4) as ptp,
            tc.tile_pool(name="ot", bufs=4) as otp,
        ):
            # ---- load persistent inputs ----
            ht = [constp.tile([128, S], f16, tag=f"ht{i}", name=f"ht{i}") for i in range(KT)]
            for i in range(KT):
                nc.sync.dma_start(ht[i][:], hT[i * 128:(i + 1) * 128, :])

            WCOL = HPC * D
            wq_all = constp.tile([128, KT * WCOL], f16, tag="wqa")
            wk_all = constp.tile([128, KT * WCOL], f16, tag="wka")
            wv_all = constp.tile([128, KT * WCOL], f16, tag="wva")
            bqk_sb = constp.tile([128, 4], f32, tag="bqk")
            nc.scalar.dma_start(bqk_sb[:], bqk[:, :])
            nc.scalar.dma_start(wq_all[:], wq[:, :])
            nc.scalar.dma_start(wk_all[:], wk[:, :])
            nc.scalar.dma_start(wv_all[:], wv[:, :])
            wq_sb = [wq_all[:, i * WCOL:(i + 1) * WCOL] for i in range(KT)]
            wk_sb = [wk_all[:, i * WCOL:(i + 1) * WCOL] for i in range(KT)]
            wv_sb = [wv_all[:, i * WCOL:(i + 1) * WCOL] for i in range(KT)]
            bq_sb = bqk_sb[:, 0:1]
            bq_sb2 = bqk_sb[0:64, 1:2]
            bk_sb = bqk_sb[:, 2:3]
            bk_sb2 = bqk_sb[0:64, 3:4]
            # Q/K in [d, s] layout: heads 0,1 in tile0 (partitions 0-63 /
            # 64-127); head 2 on partitions 0-63 of q1/k1, duplicated to
            # partitions 64-127 for row-tiled pairing.
            q0 = constp.tile([128, S], f16, tag="q0")
            q1 = constp.tile([128, S], f16, tag="q1")
            k0 = constp.tile([128, S], f16, tag="k0")
            k1 = constp.tile([128, S], f16, tag="k1")
            # V' blocks per st: [h0 d0..63, one, h1 d0..63, one, h2 ...];
            # the ones come from the memset and give the softmax row-sums.
            vp = constp.tile([128, SKT * VST], f16, tag="vp")
            warm = constp.tile([128, 512], f16, tag="warm")
            nc.vector.memset(warm[:], 0.5)
            nc.vector.memset(vp[:], 1.0)

            # ---- phase 1a: qT / kT projection (+ bias via ScalarE) ----
            with tc.tile_pool(name="psA", bufs=3, space="PSUM") as psA:
                # PE warmup during the input-DMA window: keeps the HAM clock
                # gate at 8/8 so projection and attention run at 2.4 GHz
                for w in range(19):
                    wps = psA.tile([128, 512], f32, tag="psA128", name=f"wu{w}")
                    nc.tensor.matmul(
                        wps[:], warm[:, 0:128], warm[:],
                        start=True, stop=True)
                for (dst, wsb, bsb, col0, m) in (
                    (q0, wq_sb, bq_sb, 0, 128),
                    (q1, wq_sb, bq_sb2, 128, 64),
                    (k0, wk_sb, bk_sb, 0, 128),
                    (k1, wk_sb, bk_sb2, 128, 64),
                ):
                    for c in range(SQC):
                        ps = psA.tile([m, 512], f32, tag=f"psA{m}", name=f"psA{m}")
                        for i in range(KT):
                            nc.tensor.matmul(
                                ps[:],
                                wsb[i][:, col0:col0 + m],
                                ht[i][:, c * 512:(c + 1) * 512],
                                start=(i == 0), stop=(i == KT - 1),
                            )
                        nc.vector.tensor_scalar_add(
                            dst[0:m, c * 512:(c + 1) * 512], ps[:], bsb)
                # duplicate head-2 q/k onto partitions 64-127 (SBUF->SBUF)
                nc.sync.dma_start(q1[64:128, :], q1[0:64, :])
                nc.sync.dma_start(k1[64:128, :], k1[0:64, :])

            # ---- phase 2: attention, paired QK, software-pipelined ----
            # V projection is interleaved into phase A's idle PE slots.
            def sqoff(cp, c2):
                return cp * 1024 + c2 * 512

            with (
                tc.tile_pool(name="ps", bufs=2, space="PSUM") as psp,
                tc.tile_pool(name="po", bufs=2, space="PSUM") as pop,
                tc.tile_pool(name="psV", bufs=2, space="PSUM") as psVp,
            ):
                # ---------- phase A: heads 0 and 1 ----------
                NIT = 2 * 2 * SKT     # 64 iterations (cp, c2, st)
                bt_tiles = {}
                pt_tiles = {}
                po_t = {}

                def bias_dma_A(i):
                    if i >= NIT:
                        return
                    cpc2, st = divmod(i, SKT)
                    cp, c2 = divmod(cpc2, 2)
                    bt = biasp.tile([128, 1024], i16, tag="bt", name=f"btA{i}")
                    col = cp * 2048 + c2 * 1024
                    nc.sync.dma_start(
                        bt[:], biasA[st * 128:(st + 1) * 128, col:col + 1024])
                    bt_tiles[i] = bt

                def qk_evac_A(i):
                    cpc2, st = divmod(i, SKT)
                    cp, c2 = divmod(cpc2, 2)
                    bias_dma_A(i + PRE)
                    sq = sqoff(cp, c2)
                    ps = psp.tile([128, 1024], f32, tag="ps", name=f"psA2_{i}")
                    pt = ptp.tile([128, 1024], f16, tag="pt", name=f"ptA{i}")
                    pt_tiles[i] = pt
                    nc.tensor.matmul(
                        ps[:, 0:512], k0[0:64, st * 128:(st + 1) * 128],
                        q0[0:64, sq:sq + 512], start=True, stop=True)
                    nc.tensor.matmul(
                        ps[:, 512:1024], k0[64:128, st * 128:(st + 1) * 128],
                        q0[64:128, sq:sq + 512], start=True, stop=True)
                    nc.scalar.activation(pt[:], ps[:], EXP, scale=1.0 / A_EXP)
                    nc.vector.tensor_mul(
                        pt[:], pt[:], bt_tiles.pop(i)[:].bitcast(f16))

                def pv_A(i):
                    cpc2, st = divmod(i, SKT)
                    cp, c2 = divmod(cpc2, 2)
                    if st == 0:
                        po_t[cpc2] = [
                            pop.tile([D + 1, 512], f32, tag="po",
                                     name=f"poA{cpc2}_{h}")
                            for h in range(2)]
                    po = po_t[cpc2]
                    pt = pt_tiles.pop(i)
                    for h in range(2):
                        nc.tensor.matmul(
                            po[h][:],
                            vp[:, st * VST + h * 65: st * VST + h * 65 + D + 1],
                            pt[:, h * 512:(h + 1) * 512],
                            start=(st == 0), stop=(st == SKT - 1))
                    if st == SKT - 1:
                        for h in range(2):
                            ot = otp.tile([D + 1, 512], f32, tag="ot", name="ot")
                            nc.vector.tensor_copy(ot[:], po[h][:])
                            nc.sync.dma_start(
                                out[h, :, sqoff(cp, c2):sqoff(cp, c2) + 512],
                                ot[:])

                def v_proj(st):
                    # one V-projection packet, slotted into PE idle time
                    psv = psVp.tile([128, HPC * D], f32, tag="psV", name="psV")
                    for i in range(KT):
                        nc.tensor.matmul(
                            psv[:],
                            ht[i][:, st * 128:(st + 1) * 128],
                            wv_sb[i],
                            start=(i == 0), stop=(i == KT - 1),
                        )
                    nc.vector.tensor_copy(
                        vp[:, st * VST: st * VST + VST].rearrange(
                            "p (j d) -> p j d", j=HPC)[:, :, 0:D],
                        psv[:].rearrange("p (j d) -> p j d", j=HPC))

                for i in range(PRE):
                    bias_dma_A(i)
                for i0 in range(0, NIT + LAG, 2):
                    g = i0 // 2
                    for di in range(2):
                        if i0 + di < NIT:
                            qk_evac_A(i0 + di)
                    for vs in (2 * g, 2 * g + 1):
                        if vs < SKT:
                            v_proj(vs)
                    for di in range(2):
                        ip = i0 + di - LAG
                        if 0 <= ip < NIT:
                            pv_A(ip)

                # ---------- phase B: head 2 (paired with itself) ----------
                NIT2 = 2 * 2 * (SKT // 2)   # 32 iterations (cp, c2, stp)
                bt2_tiles = {}
                pt2_tiles = {}
                po2_t = {}

                def bias_dma_B(i):
                    if i >= NIT2:
                        return
                    cpc2, stp = divmod(i, SKT // 2)
                    cp, c2 = divmod(cpc2, 2)
                    bt = biasp.tile([128, 1024], i16, tag="bt", name=f"btB{i}")
                    col = cp * 2048 + c2 * 1024
                    nc.sync.dma_start(
                        bt[:], biasB[stp * 128:(stp + 1) * 128, col:col + 1024])
                    bt2_tiles[i] = bt

                def qk_evac_B(i):
                    cpc2, stp = divmod(i, SKT // 2)
                    cp, c2 = divmod(cpc2, 2)
                    bias_dma_B(i + PRE)
                    sq = sqoff(cp, c2)
                    st0, st1 = 2 * stp, 2 * stp + 1
                    ps = psp.tile([128, 1024], f32, tag="ps", name=f"psB2_{i}")
                    pt = ptp.tile([128, 1024], f16, tag="pt", name=f"ptB{i}")
                    pt2_tiles[i] = pt
                    nc.tensor.matmul(
                        ps[:, 0:512], k1[0:64, st0 * 128:(st0 + 1) * 128],
                        q1[0:64, sq:sq + 512], start=True, stop=True)
                    nc.tensor.matmul(
                        ps[:, 512:1024], k1[64:128, st1 * 128:(st1 + 1) * 128],
                        q1[64:128, sq:sq + 512], start=True, stop=True)
                    nc.scalar.activation(pt[:], ps[:], EXP, scale=1.0 / A_EXP)
                    nc.vector.tensor_mul(
                        pt[:], pt[:], bt2_tiles.pop(i)[:].bitcast(f16))

                def pv_B(i):
                    cpc2, stp = divmod(i, SKT // 2)
                    cp, c2 = divmod(cpc2, 2)
                    if stp == 0:
                        po2_t[cpc2] = pop.tile(
                            [D + 1, 512], f32, tag="po", name=f"poB{cpc2}")
                    po = po2_t[cpc2]
                    pt = pt2_tiles.pop(i)
                    for par in range(2):
                        st = 2 * stp + par
                        nc.tensor.matmul(
                            po[:],
                            vp[:, st * VST + 2 * 65: st * VST + 2 * 65 + D + 1],
                            pt[:, par * 512:(par + 1) * 512],
                            start=(stp == 0 and par == 0),
                            stop=(stp == SKT // 2 - 1 and par == 1))
                    if stp == SKT // 2 - 1:
                        ot = otp.tile([D + 1, 512], f32, tag="ot", name="ot")
                        nc.vector.tensor_copy(ot[:], po[:])
                        nc.scalar.dma_start(
                            out[2, :, sqoff(cp, c2):sqoff(cp, c2) + 512],
                            ot[:])

                for i in range(PRE):
                    bias_dma_B(i)
                for i0 in range(0, NIT2 + LAG, 2):
                    for di in range(2):
                        if i0 + di < NIT2:
                            qk_evac_B(i0 + di)
                    for di in range(2):
                        ip = i0 + di - LAG
                        if 0 <= ip < NIT2:
                            pv_B(ip)

    nc.compile()
    return nc


def _get_nc(variant=None):
    if "nc" not in _CACHE:
        _CACHE["nc"] = _build_nc()
    return _CACHE["nc"]


def _make_in_maps(hidden_states, Wqkv_w, Wqkv_b, bias, indices, variant=None):
    hidden_states = np.asarray(hidden_states, dtype=np.float32)
    Wqkv_w = np.asarray(Wqkv_w, dtype=np.float32)
    Wqkv_b = np.asarray(Wqkv_b, dtype=np.float32)
    bias = np.asarray(bias, dtype=np.float32)
    indices = np.asarray(indices, dtype=np.int64)

    qscale = np.float32(A_EXP / math.sqrt(D))
    padded = np.zeros((TOTAL, DIM), dtype=np.float32)
    padded[indices] = hidden_states

    Wq, Wk, Wv = Wqkv_w[0:DIM], Wqkv_w[DIM:2 * DIM], Wqkv_w[2 * DIM:3 * DIM]
    bq_full = Wqkv_b[0:DIM] * qscale
    bk_full = Wqkv_b[DIM:2 * DIM]

    def pack_w(WT):  # [768, 192] -> [128, 6*192]
        return np.ascontiguousarray(
            WT.reshape(KT, 128, HPC * D).transpose(1, 0, 2).reshape(
                128, KT * HPC * D))

    in_maps = []
    for cidx in range(N_CORES):
        b = cidx // 4
        h0 = (cidx % 4) * HPC
        r = slice(h0 * D, (h0 + HPC) * D)
        bias_c = np.ascontiguousarray(bias[b, h0:h0 + HPC].transpose(0, 2, 1))
        expb = np.exp(bias_c).astype(np.float16).view(np.int16)  # [3, sk, sq]
        # biasA: [sk 2048, (cp 2, c2 2, h 2, x 512)]
        bA = expb[0:2].reshape(2, S, 2, 2, 512).transpose(1, 2, 3, 0, 4)
        bA = np.ascontiguousarray(bA.reshape(S, 2 * S))
        # biasB: [stp*128+p, (cp 2, c2 2, par 2, x 512)]
        bB = expb[2].reshape(8, 2, 128, 2, 2, 512).transpose(0, 2, 3, 4, 1, 5)
        bB = np.ascontiguousarray(bB.reshape(S // 2, 2 * S))
        bqk_arr = np.zeros((128, 4), dtype=np.float32)
        bqk_arr[:, 0] = bq_full[r][0:128]
        bqk_arr[0:64, 1] = bq_full[r][128:192]
        bqk_arr[:, 2] = bk_full[r][0:128]
        bqk_arr[0:64, 3] = bk_full[r][128:192]
        in_maps.append({
            "hT": padded[b * S:(b + 1) * S].T.astype(np.float16),
            "wq": pack_w((Wq[r].T * qscale).astype(np.float16)),
            "wk": pack_w(Wk[r].T.astype(np.float16)),
            "wv": pack_w(Wv[r].T.astype(np.float16)),
            "bqk": bqk_arr,
            "biasA": bA,
            "biasB": bB,
        })
    return in_maps


def _assemble(results, Wqkv_b, indices):
    Wqkv_b = np.asarray(Wqkv_b, dtype=np.float32)
    indices = np.asarray(indices, dtype=np.int64)
    bv = Wqkv_b[2 * DIM:3 * DIM]
    out_full = np.empty((TOTAL, DIM), dtype=np.float32)
    for c in range(N_CORES):
        b = c // 4
        h0 = (c % 4) * HPC
        o = np.asarray(results[c]["out"], dtype=np.float32)  # (3, 65, 2048)
        for j in range(HPC):
            h = h0 + j
            att = (o[j, :D] / o[j, D]).T + bv[h * D:(h + 1) * D]
            out_full[b * S:(b + 1) * S, h * D:(h + 1) * D] = att
    return out_full[indices]


VARIANT = "v5"


def kernel(hidden_states, Wqkv_w, Wqkv_b, bias, slopes, cu_seqlens, indices,
           attn_mask, max_seqlen, **_unused):
    from concourse.bass_utils import run_bass_kernel_spmd

    nc = _get_nc()
    in_maps = _make_in_maps(hidden_states, Wqkv_w, Wqkv_b, bias, indices)
    res = run_bass_kernel_spmd(nc, in_maps, list(range(N_CORES)))
    return _assemble(res.results, Wqkv_b, indices)
